# revision 9
# baseline (speedup 1.0000x reference)
"""ST-GCN autoencoder (nn_GCAE) forward pass on 8 Trainium2 NeuronCores.

Strategy: pure data parallel over batch (64 samples/core). On-chip layout
keeps (joint v, channel c) on SBUF partitions and (sample n, frame t) on the
free dim. The spatial graph conv (gcn) and adjacency multiply are folded into
one block matrix ("Wbig") contracting (c_in, v) -> (c_out, w); the temporal
conv runs as 9 shifted block-diagonal matmuls accumulating in PSUM with
edge-trimmed ranges (no zero padding needed); residual paths accumulate into
the same PSUM via extra matmuls. BN is folded into weights/biases on the
host; PSUM eviction fuses bias+ReLU on the scalar engine.
"""

import os
import numpy as np
import ml_dtypes

import concourse.bacc as bacc
import concourse.mybir as mybir
import concourse.tile as tile
from concourse.bass_utils import run_bass_kernel_spmd

F32 = mybir.dt.float32
F32R = mybir.dt.float32r
BF16 = mybir.dt.bfloat16

K = 3
V = 18
T0 = 12
NBATCH = 512
NCORES = 8
NB = NBATCH // NCORES  # 64 samples per core
EPS_SCALE = 1.0 / np.sqrt(np.float32(1.0 + 1e-5))

ENC_CFG = [(3, 32, 1, False, True), (32, 32, 1, True, True), (32, 32, 2, True, True),
           (32, 48, 1, True, True), (48, 48, 1, True, True), (48, 48, 3, True, True),
           (48, 64, 1, True, True), (64, 64, 1, True, True), (64, 32, 1, True, False)]
DEC_CONV = [(32, 64), (64, 64), (64, 48), (48, 48), (48, 48)]

# jax.image.resize('bilinear') as explicit matrices (half-pixel centers, edge clamp)
def _resize_matrix(t_out, t_in):
    R = np.zeros((t_out, t_in), np.float64)
    for tp in range(t_out):
        src = (tp + 0.5) * t_in / t_out - 0.5
        lo = int(np.floor(src))
        frac = src - lo
        for idx, wgt in ((lo, 1 - frac), (lo + 1, frac)):
            idx = min(max(idx, 0), t_in - 1)
            R[tp, idx] += wgt
    return R

R3 = _resize_matrix(6, 2)
R2 = _resize_matrix(12, 6)


def layout(C, xin=False):
    """Partition layout for channel count C: returns (stride, tiles) where
    tiles = [(v0, nv, P)] and channel c of joint v sits at partition
    (v - v0) * stride + c of its tile."""
    if xin:
        return 3, [(0, 18, 54)]
    s = 64 if C >= 48 else 32
    g = 128 // s
    tiles = []
    v0 = 0
    while v0 < V:
        nv = min(g, V - v0)
        tiles.append((v0, nv, nv * s))
        v0 += nv
    return s, tiles


def build_blocks():
    """Static per-block config (shapes/dtypes only)."""
    blocks = []
    T = T0
    cfgs = []
    for i, (ci, co, st, res, oact) in enumerate(ENC_CFG):
        cfgs.append(dict(kind='conv', src=('enc', i), cin=ci, cout=co, stride=st,
                         residual=res, out_act=oact, out_bn=True))
    cfgs.append(dict(kind='up', scale=3))
    dec_ops = ['conv', 'conv', 'conv', 'up2', 'conv', 'conv']
    ci_dec = 0
    # DEC_OPS = [up3, conv, conv, conv, up2, conv, conv]; up3 already added
    for op in ['conv', 'conv', 'conv', 'up', 'conv', 'conv']:
        if op == 'up':
            cfgs.append(dict(kind='up', scale=2))
        else:
            ci, co = DEC_CONV[ci_dec]
            cfgs.append(dict(kind='conv', src=('dec', ci_dec), cin=ci, cout=co,
                             stride=1, residual=True, out_act=True, out_bn=True))
            ci_dec += 1
    cfgs.append(dict(kind='conv', src=('final', 0), cin=48, cout=3, stride=1,
                     residual=False, out_act=False, out_bn=False))

    for b, c in enumerate(cfgs):
        if c['kind'] == 'up':
            c['T_in'] = T
            T = T * c['scale']
            c['T_out'] = T
        else:
            c['T_in'] = T
            T = T // c['stride']
            c['T_out'] = T
            c['xin'] = (b == 0)
            # residual type
            if c['residual']:
                c['res'] = 'conv' if (c['cin'] != c['cout'] or c['stride'] != 1) else 'id'
            else:
                c['res'] = None
        blocks.append(c)

    # dtype schedule: blocks with free dim < 256 for their matmuls go bf16
    for b, c in enumerate(blocks):
        if c['kind'] != 'conv':
            continue
        c['gcn_dt'] = F32R
        c['res_dt'] = F32R
        c['act_out_dt'] = F32R
    # enc indices in blocks list: 0..8. enc6=idx5 (T_out=2), enc7..9=idx 6,7,8
    for idx in (6, 7, 8):
        blocks[idx]['gcn_dt'] = BF16
        blocks[idx]['res_dt'] = BF16
    for idx in (5, 6, 7, 8):
        blocks[idx]['act_out_dt'] = BF16
    blocks[-1]['act_out_dt'] = F32  # final output, nothing consumes it on PE
    return blocks


BLOCKS = build_blocks()


def chain_layout(nblocks):
    lay = layout(3, xin=True)
    T = T0
    for b in range(nblocks):
        c = BLOCKS[b]
        if c['kind'] == 'conv':
            lay = layout(c['cout'])
            T = c['T_out']
        else:
            T = c['T_out']
    return lay, T


def _fold_cc(Wk, Aef, out_scale, in_lay, out_lay, cin, cout, in_scale=None):
    """Build packed call-blocks for a (c,v)->(c,w) contraction.

    Wk: (Kdim, cout, cin); Aef: (Kdim, V, V); out_scale: (cout,) or None.
    Returns (calls, packed) where calls = [(i, j, col_ofs, ncols)] and
    packed = [128, total_cols] float32; lhsT block for (i,j) is
    packed[0:P_i, col_ofs:col_ofs+P_j].
    """
    s_in, in_tiles = in_lay
    s_out, out_tiles = out_lay
    # F[ci, v, co, w]
    F = np.einsum('koc,kvw->cvow', Wk.astype(np.float64), Aef.astype(np.float64))
    if out_scale is not None:
        F = F * out_scale.astype(np.float64)[None, None, :, None]
    if in_scale is not None:  # in_scale[v, ci]
        F = F * in_scale.astype(np.float64).T[:, :, None, None]
    calls = []
    cols = []
    ofs = 0
    for j, (w0, nw, Pj) in enumerate(out_tiles):
        for i, (v0, nv, Pi) in enumerate(in_tiles):
            M = np.zeros((Pi, Pj), np.float64)
            for vi in range(nv):
                for wj in range(nw):
                    blk = F[:, v0 + vi, :, w0 + wj]  # (cin, cout)
                    M[vi * s_in: vi * s_in + cin, wj * s_out: wj * s_out + cout] = blk
            if np.abs(M).max() == 0.0:
                continue
            calls.append((i, j, ofs, Pj))
            cols.append(M)
            ofs += Pj
    if cols:
        packed = np.zeros((128, ofs), np.float32)
        c0 = 0
        for M in cols:
            packed[0:M.shape[0], c0:c0 + M.shape[1]] = M
            c0 += M.shape[1]
    else:
        packed = np.zeros((128, 0), np.float32)
    return calls, packed


def prep_weights(A, params):
    """Fold BN into weights, build all packed DRAM arrays. Returns dict of
    np arrays plus per-block call metadata (stored into BLOCKS)."""
    A = np.asarray(A, np.float64)
    dram = {}
    g_d = np.asarray(params['data_bn_g'], np.float64) * EPS_SCALE  # (V*C,)
    b_d = np.asarray(params['data_bn_b'], np.float64)
    s_data = g_d.reshape(V, 3)   # [v, ci]
    b_data = b_d.reshape(V, 3)

    for b, c in enumerate(BLOCKS):
        if c['kind'] != 'conv':
            continue
        src, idx = c['src']
        if src == 'enc':
            p = params['enc'][idx]
            ei = np.asarray(params['ei_enc'][idx], np.float64)
        elif src == 'dec':
            p = params['dec'][idx]
            # dec conv blocks map to DEC_OPS positions 1,2,3,5,6
            ei_idx = [1, 2, 3, 5, 6][idx]
            ei = np.asarray(params['ei_dec'][ei_idx], np.float64)
        else:
            p = params['final']
            ei = np.asarray(params['ei_dec'][-1], np.float64)
        Aef = A * ei
        cin, cout = c['cin'], c['cout']
        in_lay_full = layout(cin, xin=c.get('xin', False))
        out_lay_full = layout(cout)
        c['in_lay'] = in_lay_full
        c['out_lay'] = out_lay_full

        Wg = np.asarray(p['gcn_w'], np.float64).reshape(K, cout, cin)
        bg = np.asarray(p['gcn_b'], np.float64).reshape(K, cout)
        s1 = np.asarray(p['bn1_g'], np.float64) * EPS_SCALE
        b1 = np.asarray(p['bn1_b'], np.float64)

        in_scale = s_data if c.get('xin') else None
        calls, packed = _fold_cc(Wg, Aef, s1, in_lay_full, out_lay_full, cin, cout,
                                 in_scale=in_scale)
        c['gcn_calls'] = calls
        np_dt = np.float32 if c['gcn_dt'] != BF16 else ml_dtypes.bfloat16
        dram[f'wbig{b}'] = packed.astype(np_dt)

        # gcn bias: [co, w] = s1*(sum_k bg + data_bn bias fold) + b1
        G = np.einsum('koc,kvw->cvow', Wg, Aef)  # unscaled
        bias_cw = np.einsum('ko,kvw->ow', bg, Aef)
        if c.get('xin'):
            bias_cw = bias_cw + np.einsum('cvow,vc->ow', G, b_data)
        bias_cw = s1[:, None] * bias_cw + b1[:, None]
        s_out, out_tiles = out_lay_full
        bias_arr = np.zeros((128, len(out_tiles)), np.float32)
        for j, (w0, nw, Pj) in enumerate(out_tiles):
            for wj in range(nw):
                bias_arr[wj * s_out: wj * s_out + cout, j] = bias_cw[:, w0 + wj]
        dram[f'bias{b}'] = bias_arr

        # tcn: block-diagonal per-dt weights [128, 9*128]
        Wt = np.asarray(p['tcn_w'], np.float64)[:, :, :, 0]  # (cout, cout, 9)
        bt = np.asarray(p['tcn_b'], np.float64)
        if c['out_bn']:
            s2 = np.asarray(p['bn2_g'], np.float64) * EPS_SCALE
            b2 = np.asarray(p['bn2_b'], np.float64)
        else:
            s2 = np.ones(cout)
            b2 = np.zeros(cout)
        beta = s2 * bt + b2
        wt_arr = np.zeros((128, 9 * 128), np.float64)
        g_out = 128 // s_out
        for dt in range(9):
            for m in range(g_out):
                o = m * s_out
                # lhsT[ci, co] = s2[co] * Wt[co, ci, dt]
                wt_arr[o:o + cout, dt * 128 + o: dt * 128 + o + cout] = \
                    (s2[:, None] * Wt[:, :, dt]).T
        dram[f'wtcn{b}'] = wt_arr.astype(ml_dtypes.bfloat16)

        # residual
        if c['res'] == 'conv':
            Wr = np.asarray(p['res_w'], np.float64)[None]  # (1, cout, cin)
            br = np.asarray(p['res_b'], np.float64)
            sr = np.asarray(p['res_bn_g'], np.float64) * EPS_SCALE
            tr = np.asarray(p['res_bn_b'], np.float64)
            calls_r, packed_r = _fold_cc(Wr, np.eye(V)[None], sr,
                                         in_lay_full, out_lay_full, cin, cout)
            c['res_calls'] = calls_r
            np_dtr = np.float32 if c['res_dt'] != BF16 else ml_dtypes.bfloat16
            dram[f'wres{b}'] = packed_r.astype(np_dtr)
            beta = beta + sr * br + tr
        elif c['res'] == 'id':
            Wr = np.eye(cout)[None]
            calls_r, packed_r = _fold_cc(Wr, np.eye(V)[None], None,
                                         in_lay_full, out_lay_full, cin, cout)
            c['res_calls'] = calls_r
            np_dtr = np.float32 if c['res_dt'] != BF16 else ml_dtypes.bfloat16
            dram[f'wres{b}'] = packed_r.astype(np_dtr)

        beta_arr = np.zeros((128, len(out_tiles)), np.float32)
        for j, (w0, nw, Pj) in enumerate(out_tiles):
            for wj in range(nw):
                beta_arr[wj * s_out: wj * s_out + cout, j] = beta
        dram[f'beta{b}'] = beta_arr
    return dram


_CACHE = {}


def build_bass(dram):
    nc = bacc.Bacc()
    nblocks = int(os.environ.get('KB_NUM_BLOCKS', len(BLOCKS)))
    dram_shapes = {k: (v.shape, v.dtype) for k, v in dram.items()}

    x_d = nc.dram_tensor('xin', [54, NB * T0], F32R, kind='ExternalInput')
    w_d = {}
    for name, arr in dram.items():
        dt = {np.dtype(np.float32): F32, np.dtype(ml_dtypes.bfloat16): BF16}[np.dtype(arr.dtype)]
        if name.startswith('wbig') or name.startswith('wres'):
            if dt == F32:
                dt = F32R
        w_d[name] = nc.dram_tensor(name, list(arr.shape), dt, kind='ExternalInput')

    if nblocks == len(BLOCKS):
        y_d = nc.dram_tensor('y', [54, NB * T0], F32, kind='ExternalOutput')
    else:
        (s_o, t_o), T_fin = chain_layout(nblocks)
        y_d = nc.dram_tensor('y', [len(t_o) * 128, NB * T_fin], F32,
                             kind='ExternalOutput')

    tc_obj = tile.TileContext(nc, trace_sim=bool(int(os.environ.get('KB_SIMTRACE', '0'))))
    with tc_obj as tc:
        with (
            tc.tile_pool(name='consts', bufs=1) as consts,
            tc.tile_pool(name='wpool', bufs=1) as wpool,
            tc.tile_pool(name='acts', bufs=1) as acts,
            tc.tile_pool(name='uptmps', bufs=6) as uptmps,
            tc.tile_pool(name='psum', bufs=4, space='PSUM') as psum,
        ):
            xin = consts.tile([54, NB, T0], F32R)
            nc.gpsimd.dma_start(xin[:], x_d[:].rearrange('p (n t) -> p n t', t=T0))

            cur = [xin]           # list of act tiles
            cur_lay = layout(3, xin=True)
            cur_T = T0

            for b in range(nblocks):
                c = BLOCKS[b]
                if c['kind'] == 'up':
                    R = R3 if c['scale'] == 3 else R2
                    T_in, T_out = c['T_in'], c['T_out']
                    s_l, tiles_l = cur_lay
                    out_tiles = []
                    for i, (v0, nv, Pi) in enumerate(tiles_l):
                        ot = acts.tile([128, NB, T_out], F32R, tag=f'act{b % 2}_{i}', name=f'up{b}_{i}')
                        src = cur[i]
                        if src.dtype == F32R:
                            src = src[:, :, :].bitcast(F32)
                        for tp in range(T_out):
                            nzs = [(ti, R[tp, ti]) for ti in range(T_in) if R[tp, ti] != 0.0]
                            if len(nzs) == 1:
                                ti, wgt = nzs[0]
                                if wgt == 1.0:
                                    nc.vector.tensor_copy(ot[0:Pi, :, tp], src[0:Pi, :, ti])
                                else:
                                    nc.vector.tensor_scalar_mul(
                                        ot[0:Pi, :, tp], src[0:Pi, :, ti], float(wgt))
                            else:
                                (t0, w0), (t1, w1) = nzs[0], nzs[1]
                                tmp = uptmps.tile([128, NB], F32, tag='uptmp', name=f'uptmp{b}_{i}_{tp}')
                                nc.vector.tensor_scalar_mul(
                                    tmp[0:Pi], src[0:Pi, :, t1], float(w1))
                                nc.vector.scalar_tensor_tensor(
                                    ot[0:Pi, :, tp], src[0:Pi, :, t0], float(w0),
                                    tmp[0:Pi], mybir.AluOpType.mult, mybir.AluOpType.add)
                        out_tiles.append(ot)
                    cur = out_tiles
                    cur_T = T_out
                    continue

                # conv block
                T_in, T_out, stride = c['T_in'], c['T_out'], c['stride']
                s_in, in_tiles = c['in_lay']
                s_out, out_tiles_l = c['out_lay']
                n_out = len(out_tiles_l)
                gdt = c['gcn_dt']
                rdt = c['res_dt']
                odt = c['act_out_dt']

                wbig_sh = dram_shapes[f'wbig{b}'][0]
                wbig = wpool.tile([128, max(wbig_sh[1], 1)], gdt, tag=f'wbig{b % 2}')
                if wbig_sh[1] > 0:
                    nc.gpsimd.dma_start(wbig[:, 0:wbig_sh[1]], w_d[f'wbig{b}'][:])
                wtcn = wpool.tile([128, 9 * 128], BF16, tag=f'wtcn{b % 2}')
                nc.gpsimd.dma_start(wtcn[:], w_d[f'wtcn{b}'][:])
                bias_sb = wpool.tile([128, n_out], F32, tag=f'bias{b % 2}')
                nc.gpsimd.dma_start(bias_sb[:], w_d[f'bias{b}'][:])
                beta_sb = wpool.tile([128, n_out], F32, tag=f'beta{b % 2}')
                nc.gpsimd.dma_start(beta_sb[:], w_d[f'beta{b}'][:])
                if c['res'] is not None:
                    wres_sh = dram_shapes[f'wres{b}'][0]
                    wres = wpool.tile([128, max(wres_sh[1], 1)], rdt, tag=f'wres{b % 2}')
                    nc.gpsimd.dma_start(wres[:, 0:wres_sh[1]], w_d[f'wres{b}'][:])

                nchunk = 32 if max(T_in, T_out) >= 12 else NB
                nck = NB // nchunk

                gact = [acts.tile([128, NB, T_in], BF16, tag=f'gact_{i}', name=f'gact{b}_{i}')
                        for i in range(n_out)]
                oact = [acts.tile([128, NB, T_out], odt, tag=f'act{b % 2}_{i}', name=f'oact{b}_{i}')
                        for i in range(n_out)]

                # group gcn calls by output tile
                by_j = {}
                for (i, j, ofs, ncols) in c['gcn_calls']:
                    by_j.setdefault(j, []).append((i, ofs))
                res_by_j = {}
                if c['res'] is not None:
                    for (i, j, ofs, ncols) in c['res_calls']:
                        res_by_j.setdefault(j, []).append((i, ofs))

                for ck in range(nck):
                    n0, n1 = ck * nchunk, (ck + 1) * nchunk
                    # --- gcn (+A, +bn1, +relu) ---
                    for j in range(n_out):
                        w0j, nwj, Pj = out_tiles_l[j]
                        ps = psum.tile([128, nchunk, T_in], F32, tag='ps_g', name=f'psg{b}_{ck}_{j}')
                        lst = by_j.get(j, [])
                        for q, (i, ofs) in enumerate(lst):
                            v0i, nvi, Pi = in_tiles[i]
                            nc.tensor.matmul(
                                ps[0:Pj, :, :],
                                wbig[0:Pi, ofs:ofs + Pj],
                                cur[i][0:Pi, n0:n1, :],
                                start=(q == 0), stop=(q == len(lst) - 1))
                        nc.scalar.activation(
                            gact[j][0:Pj, n0:n1, :], ps[0:Pj, :, :],
                            mybir.ActivationFunctionType.Relu,
                            bias=bias_sb[0:Pj, j:j + 1])
                    # --- tcn (+res, +bn2, +beta, +out_act) ---
                    for j in range(n_out):
                        w0j, nwj, Pj = out_tiles_l[j]
                        ps2 = psum.tile([128, nchunk, T_out], F32, tag='ps_t', name=f'pst{b}_{ck}_{j}')
                        dts = []
                        for dt in range(9):
                            t_lo = max(0, -(-(4 - dt) // stride))
                            # smallest t' with stride*t'+dt-4 >= 0
                            t_lo = max(0, (4 - dt + stride - 1) // stride)
                            # largest t' with stride*t'+dt-4 <= T_in-1
                            t_hi = min(T_out, (T_in - 1 - dt + 4) // stride + 1)
                            if t_hi > t_lo:
                                dts.append((dt, t_lo, t_hi))
                        # center tap first covers full range (start=True)
                        dts.sort(key=lambda z: -(z[2] - z[1]))
                        assert dts[0][1] == 0 and dts[0][2] == T_out, (b, dts)
                        n_calls = len(dts) + len(res_by_j.get(j, []))
                        q = 0
                        for (dt, t_lo, t_hi) in dts:
                            in_lo = stride * t_lo + dt - 4
                            nc.tensor.matmul(
                                ps2[0:Pj, :, t_lo:t_hi],
                                wtcn[0:Pj, dt * 128: dt * 128 + Pj],
                                gact[j][0:Pj, n0:n1, in_lo: in_lo + (t_hi - t_lo - 1) * stride + 1: stride]
                                if stride > 1 else
                                gact[j][0:Pj, n0:n1, in_lo: in_lo + (t_hi - t_lo)],
                                start=(q == 0), stop=(q == n_calls - 1))
                            q += 1
                        for (i, ofs) in res_by_j.get(j, []):
                            v0i, nvi, Pi = in_tiles[i]
                            rhs = cur[i][0:Pi, n0:n1, ::stride] if stride > 1 \
                                else cur[i][0:Pi, n0:n1, :]
                            nc.tensor.matmul(
                                ps2[0:Pj, :, :], wres[0:Pi, ofs:ofs + Pj], rhs,
                                start=False, stop=(q == n_calls - 1))
                            q += 1
                        func = (mybir.ActivationFunctionType.Relu if c['out_act']
                                else mybir.ActivationFunctionType.Identity)
                        nc.scalar.activation(
                            oact[j][0:Pj, n0:n1, :], ps2[0:Pj, :, :], func,
                            bias=beta_sb[0:Pj, j:j + 1])

                cur = oact
                cur_lay = c['out_lay']
                cur_T = T_out

            # write output
            if nblocks == len(BLOCKS):
                s_o, tiles_o = cur_lay
                for v in range(V):
                    ti = None
                    for i, (v0, nv, Pi) in enumerate(tiles_o):
                        if v0 <= v < v0 + nv:
                            ti = i
                            ro = (v - v0) * s_o
                    nc.gpsimd.dma_start(
                        y_d[v * 3:(v + 1) * 3, :].rearrange('p (n t) -> p n t', t=T0),
                        cur[ti][ro:ro + 3, :, :])
            else:
                s_o, tiles_o = cur_lay
                for i, (v0, nv, Pi) in enumerate(tiles_o):
                    src = cur[i][0:128, :, :]
                    if src.dtype == F32R:
                        src = src.bitcast(F32)
                    tmp = acts.tile([128, NB, cur_T], F32, tag=f'dbgout_{i}', name=f'dbg_{i}')
                    nc.vector.tensor_copy(tmp[:], src)
                    src = tmp[:]
                    nc.gpsimd.dma_start(
                        y_d[i * 128:(i + 1) * 128, :].rearrange(
                            'p (n t) -> p n t', t=cur_T), src)

    nc._kb_perfetto = getattr(tc_obj, '_perfetto_entries', None)
    nc.compile()
    return nc


def kernel(x, A, params):
    x = np.asarray(x, np.float32)
    dram = prep_weights(A, params)
    # cache key: the sparsity call pattern + shapes (the compiled program
    # depends on which Wbig blocks are nonzero, not on the weight values)
    key = (tuple(
        (b_i, tuple(c.get('gcn_calls', ())), tuple(c.get('res_calls', ())))
        for b_i, c in enumerate(BLOCKS) if c['kind'] == 'conv'),
        tuple(sorted((k, v.shape, str(v.dtype)) for k, v in dram.items())))
    if key not in _CACHE:
        _CACHE.clear()
        _CACHE[key] = build_bass(dram)
    nc = _CACHE[key]

    # x -> per-core [54, NB*T0] layout: row v*3+c, col n*T0+t
    in_maps = []
    for core in range(NCORES):
        xc = x[core * NB:(core + 1) * NB]            # (NB, 3, 12, 18)
        xl = np.ascontiguousarray(xc.transpose(3, 1, 0, 2)).reshape(54, NB * T0)
        m = {'xin': xl.astype(np.float32)}
        m.update(dram)
        in_maps.append(m)

    import time as _time
    trace = bool(int(os.environ.get('KB_TRACE', '0')))
    _t0 = _time.time()
    res = run_bass_kernel_spmd(nc, in_maps, core_ids=list(range(NCORES)), trace=trace)
    kernel.last_run_wall_ns = int((_time.time() - _t0) * 1e9)
    if trace and res.exec_time_ns is not None:
        kernel.last_exec_time_ns = res.exec_time_ns
        kernel.last_results = res

    nblocks = int(os.environ.get('KB_NUM_BLOCKS', len(BLOCKS)))
    outs = []
    for core in range(NCORES):
        yc = res.results[core]['y']
        if nblocks == len(BLOCKS):
            # [54, NB*T0] -> (NB, 3, 12, 18)
            out = yc.reshape(18, 3, NB, T0).transpose(2, 1, 3, 0)
            outs.append(out)
        else:
            outs.append(yc)
    if nblocks == len(BLOCKS):
        return np.ascontiguousarray(np.concatenate(outs, axis=0).astype(np.float32))
    return np.stack(outs)


# revision 10
# speedup vs baseline: 1.0188x; 1.0188x over previous
"""ST-GCN autoencoder (nn_GCAE) forward pass on 8 Trainium2 NeuronCores.

Strategy: pure data parallel over batch (64 samples/core). On-chip layout
keeps (joint v, channel c) on SBUF partitions and (sample n, frame t) on the
free dim. The spatial graph conv (gcn) and adjacency multiply are folded into
one block matrix ("Wbig") contracting (c_in, v) -> (c_out, w); the temporal
conv runs as 9 shifted block-diagonal matmuls accumulating in PSUM with
edge-trimmed ranges (no zero padding needed); residual paths accumulate into
the same PSUM via extra matmuls. BN is folded into weights/biases on the
host; PSUM eviction fuses bias+ReLU on the scalar engine.
"""

import os
import numpy as np
import ml_dtypes

import concourse.bacc as bacc
import concourse.mybir as mybir
import concourse.tile as tile
from concourse.bass_utils import run_bass_kernel_spmd

F32 = mybir.dt.float32
F32R = mybir.dt.float32r
BF16 = mybir.dt.bfloat16

K = 3
V = 18
T0 = 12
NBATCH = 512
NCORES = 8
NB = NBATCH // NCORES  # 64 samples per core
EPS_SCALE = 1.0 / np.sqrt(np.float32(1.0 + 1e-5))

ENC_CFG = [(3, 32, 1, False, True), (32, 32, 1, True, True), (32, 32, 2, True, True),
           (32, 48, 1, True, True), (48, 48, 1, True, True), (48, 48, 3, True, True),
           (48, 64, 1, True, True), (64, 64, 1, True, True), (64, 32, 1, True, False)]
DEC_CONV = [(32, 64), (64, 64), (64, 48), (48, 48), (48, 48)]

# jax.image.resize('bilinear') as explicit matrices (half-pixel centers, edge clamp)
def _resize_matrix(t_out, t_in):
    R = np.zeros((t_out, t_in), np.float64)
    for tp in range(t_out):
        src = (tp + 0.5) * t_in / t_out - 0.5
        lo = int(np.floor(src))
        frac = src - lo
        for idx, wgt in ((lo, 1 - frac), (lo + 1, frac)):
            idx = min(max(idx, 0), t_in - 1)
            R[tp, idx] += wgt
    return R

R3 = _resize_matrix(6, 2)
R2 = _resize_matrix(12, 6)


def layout(C, xin=False):
    """Partition layout for channel count C: returns (stride, tiles) where
    tiles = [(v0, nv, P)] and channel c of joint v sits at partition
    (v - v0) * stride + c of its tile."""
    if xin:
        return 3, [(0, 18, 54)]
    s = 64 if C >= 48 else 32
    g = 128 // s
    tiles = []
    v0 = 0
    while v0 < V:
        nv = min(g, V - v0)
        tiles.append((v0, nv, nv * s))
        v0 += nv
    return s, tiles


def build_blocks():
    """Static per-block config (shapes/dtypes only)."""
    blocks = []
    T = T0
    cfgs = []
    for i, (ci, co, st, res, oact) in enumerate(ENC_CFG):
        cfgs.append(dict(kind='conv', src=('enc', i), cin=ci, cout=co, stride=st,
                         residual=res, out_act=oact, out_bn=True))
    cfgs.append(dict(kind='up', scale=3))
    dec_ops = ['conv', 'conv', 'conv', 'up2', 'conv', 'conv']
    ci_dec = 0
    # DEC_OPS = [up3, conv, conv, conv, up2, conv, conv]; up3 already added
    for op in ['conv', 'conv', 'conv', 'up', 'conv', 'conv']:
        if op == 'up':
            cfgs.append(dict(kind='up', scale=2))
        else:
            ci, co = DEC_CONV[ci_dec]
            cfgs.append(dict(kind='conv', src=('dec', ci_dec), cin=ci, cout=co,
                             stride=1, residual=True, out_act=True, out_bn=True))
            ci_dec += 1
    cfgs.append(dict(kind='conv', src=('final', 0), cin=48, cout=3, stride=1,
                     residual=False, out_act=False, out_bn=False))

    for b, c in enumerate(cfgs):
        if c['kind'] == 'up':
            c['T_in'] = T
            T = T * c['scale']
            c['T_out'] = T
        else:
            c['T_in'] = T
            T = T // c['stride']
            c['T_out'] = T
            c['xin'] = (b == 0)
            # residual type
            if c['residual']:
                c['res'] = 'conv' if (c['cin'] != c['cout'] or c['stride'] != 1) else 'id'
            else:
                c['res'] = None
        blocks.append(c)

    # dtype schedule: blocks with free dim < 256 for their matmuls go bf16
    for b, c in enumerate(blocks):
        if c['kind'] != 'conv':
            continue
        c['gcn_dt'] = F32R
        c['res_dt'] = F32R
        c['act_out_dt'] = F32R
    # enc indices in blocks list: 0..8. enc6=idx5 (T_out=2), enc7..9=idx 6,7,8
    for idx in (6, 7, 8):
        blocks[idx]['gcn_dt'] = BF16
        blocks[idx]['res_dt'] = BF16
    for idx in (5, 6, 7, 8):
        blocks[idx]['act_out_dt'] = BF16
    blocks[-1]['act_out_dt'] = F32  # final output, nothing consumes it on PE
    return blocks


BLOCKS = build_blocks()


def chain_layout(nblocks):
    lay = layout(3, xin=True)
    T = T0
    for b in range(nblocks):
        c = BLOCKS[b]
        if c['kind'] == 'conv':
            lay = layout(c['cout'])
            T = c['T_out']
        else:
            T = c['T_out']
    return lay, T


def _fold_cc(Wk, Aef, out_scale, in_lay, out_lay, cin, cout, in_scale=None):
    """Build packed call-blocks for a (c,v)->(c,w) contraction.

    Wk: (Kdim, cout, cin); Aef: (Kdim, V, V); out_scale: (cout,) or None.
    Returns (calls, packed) where calls = [(i, j, col_ofs, ncols)] and
    packed = [128, total_cols] float32; lhsT block for (i,j) is
    packed[0:P_i, col_ofs:col_ofs+P_j].
    """
    s_in, in_tiles = in_lay
    s_out, out_tiles = out_lay
    # F[ci, v, co, w]
    F = np.einsum('koc,kvw->cvow', Wk.astype(np.float64), Aef.astype(np.float64))
    if out_scale is not None:
        F = F * out_scale.astype(np.float64)[None, None, :, None]
    if in_scale is not None:  # in_scale[v, ci]
        F = F * in_scale.astype(np.float64).T[:, :, None, None]
    calls = []
    cols = []
    ofs = 0
    for j, (w0, nw, Pj) in enumerate(out_tiles):
        for i, (v0, nv, Pi) in enumerate(in_tiles):
            M = np.zeros((Pi, Pj), np.float64)
            for vi in range(nv):
                for wj in range(nw):
                    blk = F[:, v0 + vi, :, w0 + wj]  # (cin, cout)
                    M[vi * s_in: vi * s_in + cin, wj * s_out: wj * s_out + cout] = blk
            if np.abs(M).max() == 0.0:
                continue
            calls.append((i, j, ofs, Pj))
            cols.append(M)
            ofs += Pj
    if cols:
        packed = np.zeros((128, ofs), np.float32)
        c0 = 0
        for M in cols:
            packed[0:M.shape[0], c0:c0 + M.shape[1]] = M
            c0 += M.shape[1]
    else:
        packed = np.zeros((128, 0), np.float32)
    return calls, packed


def prep_weights(A, params):
    """Fold BN into weights, build all packed DRAM arrays. Returns dict of
    np arrays plus per-block call metadata (stored into BLOCKS)."""
    A = np.asarray(A, np.float64)
    dram = {}
    g_d = np.asarray(params['data_bn_g'], np.float64) * EPS_SCALE  # (V*C,)
    b_d = np.asarray(params['data_bn_b'], np.float64)
    s_data = g_d.reshape(V, 3)   # [v, ci]
    b_data = b_d.reshape(V, 3)

    for b, c in enumerate(BLOCKS):
        if c['kind'] != 'conv':
            continue
        src, idx = c['src']
        if src == 'enc':
            p = params['enc'][idx]
            ei = np.asarray(params['ei_enc'][idx], np.float64)
        elif src == 'dec':
            p = params['dec'][idx]
            # dec conv blocks map to DEC_OPS positions 1,2,3,5,6
            ei_idx = [1, 2, 3, 5, 6][idx]
            ei = np.asarray(params['ei_dec'][ei_idx], np.float64)
        else:
            p = params['final']
            ei = np.asarray(params['ei_dec'][-1], np.float64)
        Aef = A * ei
        cin, cout = c['cin'], c['cout']
        in_lay_full = layout(cin, xin=c.get('xin', False))
        out_lay_full = layout(cout)
        c['in_lay'] = in_lay_full
        c['out_lay'] = out_lay_full

        Wg = np.asarray(p['gcn_w'], np.float64).reshape(K, cout, cin)
        bg = np.asarray(p['gcn_b'], np.float64).reshape(K, cout)
        s1 = np.asarray(p['bn1_g'], np.float64) * EPS_SCALE
        b1 = np.asarray(p['bn1_b'], np.float64)

        in_scale = s_data if c.get('xin') else None
        calls, packed = _fold_cc(Wg, Aef, s1, in_lay_full, out_lay_full, cin, cout,
                                 in_scale=in_scale)
        c['gcn_calls'] = calls
        np_dt = np.float32 if c['gcn_dt'] != BF16 else ml_dtypes.bfloat16
        dram[f'wbig{b}'] = packed.astype(np_dt)

        # gcn bias: [co, w] = s1*(sum_k bg + data_bn bias fold) + b1
        G = np.einsum('koc,kvw->cvow', Wg, Aef)  # unscaled
        bias_cw = np.einsum('ko,kvw->ow', bg, Aef)
        if c.get('xin'):
            bias_cw = bias_cw + np.einsum('cvow,vc->ow', G, b_data)
        bias_cw = s1[:, None] * bias_cw + b1[:, None]
        s_out, out_tiles = out_lay_full
        bias_arr = np.zeros((128, len(out_tiles)), np.float32)
        for j, (w0, nw, Pj) in enumerate(out_tiles):
            for wj in range(nw):
                bias_arr[wj * s_out: wj * s_out + cout, j] = bias_cw[:, w0 + wj]
        dram[f'bias{b}'] = bias_arr

        # tcn: block-diagonal per-dt weights [128, 9*128]
        Wt = np.asarray(p['tcn_w'], np.float64)[:, :, :, 0]  # (cout, cout, 9)
        bt = np.asarray(p['tcn_b'], np.float64)
        if c['out_bn']:
            s2 = np.asarray(p['bn2_g'], np.float64) * EPS_SCALE
            b2 = np.asarray(p['bn2_b'], np.float64)
        else:
            s2 = np.ones(cout)
            b2 = np.zeros(cout)
        beta = s2 * bt + b2
        wt_arr = np.zeros((128, 9 * 128), np.float64)
        g_out = 128 // s_out
        for dt in range(9):
            for m in range(g_out):
                o = m * s_out
                # lhsT[ci, co] = s2[co] * Wt[co, ci, dt]
                wt_arr[o:o + cout, dt * 128 + o: dt * 128 + o + cout] = \
                    (s2[:, None] * Wt[:, :, dt]).T
        dram[f'wtcn{b}'] = wt_arr.astype(ml_dtypes.bfloat16)

        # residual
        if c['res'] == 'conv':
            Wr = np.asarray(p['res_w'], np.float64)[None]  # (1, cout, cin)
            br = np.asarray(p['res_b'], np.float64)
            sr = np.asarray(p['res_bn_g'], np.float64) * EPS_SCALE
            tr = np.asarray(p['res_bn_b'], np.float64)
            calls_r, packed_r = _fold_cc(Wr, np.eye(V)[None], sr,
                                         in_lay_full, out_lay_full, cin, cout)
            c['res_calls'] = calls_r
            np_dtr = np.float32 if c['res_dt'] != BF16 else ml_dtypes.bfloat16
            dram[f'wres{b}'] = packed_r.astype(np_dtr)
            beta = beta + sr * br + tr
        elif c['res'] == 'id':
            Wr = np.eye(cout)[None]
            calls_r, packed_r = _fold_cc(Wr, np.eye(V)[None], None,
                                         in_lay_full, out_lay_full, cin, cout)
            c['res_calls'] = calls_r
            np_dtr = np.float32 if c['res_dt'] != BF16 else ml_dtypes.bfloat16
            dram[f'wres{b}'] = packed_r.astype(np_dtr)

        beta_arr = np.zeros((128, len(out_tiles)), np.float32)
        for j, (w0, nw, Pj) in enumerate(out_tiles):
            for wj in range(nw):
                beta_arr[wj * s_out: wj * s_out + cout, j] = beta
        dram[f'beta{b}'] = beta_arr
    return dram


_CACHE = {}


def build_bass(dram):
    nc = bacc.Bacc()
    nblocks = int(os.environ.get('KB_NUM_BLOCKS', len(BLOCKS)))
    dram_shapes = {k: (v.shape, v.dtype) for k, v in dram.items()}

    x_d = nc.dram_tensor('xin', [54, NB * T0], F32R, kind='ExternalInput')
    w_d = {}
    for name, arr in dram.items():
        dt = {np.dtype(np.float32): F32, np.dtype(ml_dtypes.bfloat16): BF16}[np.dtype(arr.dtype)]
        if name.startswith('wbig') or name.startswith('wres'):
            if dt == F32:
                dt = F32R
        w_d[name] = nc.dram_tensor(name, list(arr.shape), dt, kind='ExternalInput')

    if nblocks == len(BLOCKS):
        y_d = nc.dram_tensor('y', [54, NB * T0], F32, kind='ExternalOutput')
    else:
        (s_o, t_o), T_fin = chain_layout(nblocks)
        y_d = nc.dram_tensor('y', [len(t_o) * 128, NB * T_fin], F32,
                             kind='ExternalOutput')

    tc_obj = tile.TileContext(nc, trace_sim=bool(int(os.environ.get('KB_SIMTRACE', '0'))))
    with tc_obj as tc:
        with (
            tc.tile_pool(name='consts', bufs=1) as consts,
            tc.tile_pool(name='wpool', bufs=1) as wpool,
            tc.tile_pool(name='acts', bufs=1) as acts,
            tc.tile_pool(name='uptmps', bufs=6) as uptmps,
            tc.tile_pool(name='psum', bufs=4, space='PSUM') as psum,
        ):
            xin = consts.tile([54, NB, T0], F32R)
            nc.gpsimd.dma_start(xin[:], x_d[:].rearrange('p (n t) -> p n t', t=T0))

            cur = [xin]           # list of act tiles
            cur_lay = layout(3, xin=True)
            cur_T = T0

            for b in range(nblocks):
                c = BLOCKS[b]
                if c['kind'] == 'up':
                    R = R3 if c['scale'] == 3 else R2
                    T_in, T_out = c['T_in'], c['T_out']
                    s_l, tiles_l = cur_lay
                    out_tiles = []
                    for i, (v0, nv, Pi) in enumerate(tiles_l):
                        ot = acts.tile([128, NB, T_out], F32R, tag=f'act{b % 2}_{i}', name=f'up{b}_{i}')
                        src = cur[i]
                        if src.dtype == F32R:
                            src = src[:, :, :].bitcast(F32)
                        for tp in range(T_out):
                            nzs = [(ti, R[tp, ti]) for ti in range(T_in) if R[tp, ti] != 0.0]
                            if len(nzs) == 1:
                                ti, wgt = nzs[0]
                                if wgt == 1.0:
                                    nc.vector.tensor_copy(ot[0:Pi, :, tp], src[0:Pi, :, ti])
                                else:
                                    nc.vector.tensor_scalar_mul(
                                        ot[0:Pi, :, tp], src[0:Pi, :, ti], float(wgt))
                            else:
                                (t0, w0), (t1, w1) = nzs[0], nzs[1]
                                tmp = uptmps.tile([128, NB], F32, tag='uptmp', name=f'uptmp{b}_{i}_{tp}')
                                nc.vector.tensor_scalar_mul(
                                    tmp[0:Pi], src[0:Pi, :, t1], float(w1))
                                nc.vector.scalar_tensor_tensor(
                                    ot[0:Pi, :, tp], src[0:Pi, :, t0], float(w0),
                                    tmp[0:Pi], mybir.AluOpType.mult, mybir.AluOpType.add)
                        out_tiles.append(ot)
                    cur = out_tiles
                    cur_T = T_out
                    continue

                # conv block
                T_in, T_out, stride = c['T_in'], c['T_out'], c['stride']
                s_in, in_tiles = c['in_lay']
                s_out, out_tiles_l = c['out_lay']
                n_out = len(out_tiles_l)
                gdt = c['gcn_dt']
                rdt = c['res_dt']
                odt = c['act_out_dt']

                wbig_sh = dram_shapes[f'wbig{b}'][0]
                wbig = wpool.tile([128, max(wbig_sh[1], 1)], gdt, tag=f'wbig{b % 2}')
                if wbig_sh[1] > 0:
                    nc.gpsimd.dma_start(wbig[:, 0:wbig_sh[1]], w_d[f'wbig{b}'][:])
                wtcn = wpool.tile([128, 9 * 128], BF16, tag=f'wtcn{b % 2}')
                nc.gpsimd.dma_start(wtcn[:], w_d[f'wtcn{b}'][:])
                bias_sb = wpool.tile([128, n_out], F32, tag=f'bias{b % 2}')
                nc.gpsimd.dma_start(bias_sb[:], w_d[f'bias{b}'][:])
                beta_sb = wpool.tile([128, n_out], F32, tag=f'beta{b % 2}')
                nc.gpsimd.dma_start(beta_sb[:], w_d[f'beta{b}'][:])
                if c['res'] is not None:
                    wres_sh = dram_shapes[f'wres{b}'][0]
                    wres = wpool.tile([128, max(wres_sh[1], 1)], rdt, tag=f'wres{b % 2}')
                    nc.gpsimd.dma_start(wres[:, 0:wres_sh[1]], w_d[f'wres{b}'][:])

                nchunk = 32 if max(T_in, T_out) >= 12 else NB
                nck = NB // nchunk

                gact = [acts.tile([128, NB, T_in], BF16, tag=f'gact_{i}', name=f'gact{b}_{i}')
                        for i in range(n_out)]
                oact = [acts.tile([128, NB, T_out], odt, tag=f'act{b % 2}_{i}', name=f'oact{b}_{i}')
                        for i in range(n_out)]

                # group gcn calls by output tile
                by_j = {}
                for (i, j, ofs, ncols) in c['gcn_calls']:
                    by_j.setdefault(j, []).append((i, ofs))
                res_by_j = {}
                if c['res'] is not None:
                    for (i, j, ofs, ncols) in c['res_calls']:
                        res_by_j.setdefault(j, []).append((i, ofs))

                for ck in range(nck):
                    n0, n1 = ck * nchunk, (ck + 1) * nchunk
                    # --- gcn (+A, +bn1, +relu) ---
                    for j in range(n_out):
                        w0j, nwj, Pj = out_tiles_l[j]
                        ps = psum.tile([128, nchunk, T_in], F32, tag='ps_g', name=f'psg{b}_{ck}_{j}')
                        lst = by_j.get(j, [])
                        for q, (i, ofs) in enumerate(lst):
                            v0i, nvi, Pi = in_tiles[i]
                            nc.tensor.matmul(
                                ps[0:Pj, :, :],
                                wbig[0:Pi, ofs:ofs + Pj],
                                cur[i][0:Pi, n0:n1, :],
                                start=(q == 0), stop=(q == len(lst) - 1))
                        nc.scalar.activation(
                            gact[j][0:Pj, n0:n1, :], ps[0:Pj, :, :],
                            mybir.ActivationFunctionType.Relu,
                            bias=bias_sb[0:Pj, j:j + 1])
                    # --- tcn (+res, +bn2, +beta, +out_act) ---
                    for j in range(n_out):
                        w0j, nwj, Pj = out_tiles_l[j]
                        ps2 = psum.tile([128, nchunk, T_out], F32, tag='ps_t', name=f'pst{b}_{ck}_{j}')
                        dts = []
                        for dt in range(9):
                            t_lo = max(0, -(-(4 - dt) // stride))
                            # smallest t' with stride*t'+dt-4 >= 0
                            t_lo = max(0, (4 - dt + stride - 1) // stride)
                            # largest t' with stride*t'+dt-4 <= T_in-1
                            t_hi = min(T_out, (T_in - 1 - dt + 4) // stride + 1)
                            if t_hi > t_lo:
                                dts.append((dt, t_lo, t_hi))
                        # center tap first covers full range (start=True)
                        dts.sort(key=lambda z: -(z[2] - z[1]))
                        assert dts[0][1] == 0 and dts[0][2] == T_out, (b, dts)
                        n_calls = len(dts) + len(res_by_j.get(j, []))
                        q = 0
                        for (dt, t_lo, t_hi) in dts:
                            in_lo = stride * t_lo + dt - 4
                            nc.tensor.matmul(
                                ps2[0:Pj, :, t_lo:t_hi],
                                wtcn[0:Pj, dt * 128: dt * 128 + Pj],
                                gact[j][0:Pj, n0:n1, in_lo: in_lo + (t_hi - t_lo - 1) * stride + 1: stride]
                                if stride > 1 else
                                gact[j][0:Pj, n0:n1, in_lo: in_lo + (t_hi - t_lo)],
                                start=(q == 0), stop=(q == n_calls - 1))
                            q += 1
                        for (i, ofs) in res_by_j.get(j, []):
                            v0i, nvi, Pi = in_tiles[i]
                            rhs = cur[i][0:Pi, n0:n1, ::stride] if stride > 1 \
                                else cur[i][0:Pi, n0:n1, :]
                            nc.tensor.matmul(
                                ps2[0:Pj, :, :], wres[0:Pi, ofs:ofs + Pj], rhs,
                                start=False, stop=(q == n_calls - 1))
                            q += 1
                        func = (mybir.ActivationFunctionType.Relu if c['out_act']
                                else mybir.ActivationFunctionType.Identity)
                        nc.scalar.activation(
                            oact[j][0:Pj, n0:n1, :], ps2[0:Pj, :, :], func,
                            bias=beta_sb[0:Pj, j:j + 1])

                cur = oact
                cur_lay = c['out_lay']
                cur_T = T_out

            # write output
            if nblocks == len(BLOCKS):
                s_o, tiles_o = cur_lay
                for v in range(V):
                    ti = None
                    for i, (v0, nv, Pi) in enumerate(tiles_o):
                        if v0 <= v < v0 + nv:
                            ti = i
                            ro = (v - v0) * s_o
                    nc.gpsimd.dma_start(
                        y_d[v * 3:(v + 1) * 3, :].rearrange('p (n t) -> p n t', t=T0),
                        cur[ti][ro:ro + 3, :, :])
            else:
                s_o, tiles_o = cur_lay
                for i, (v0, nv, Pi) in enumerate(tiles_o):
                    src = cur[i][0:128, :, :]
                    if src.dtype == F32R:
                        src = src.bitcast(F32)
                    tmp = acts.tile([128, NB, cur_T], F32, tag=f'dbgout_{i}', name=f'dbg_{i}')
                    nc.vector.tensor_copy(tmp[:], src)
                    src = tmp[:]
                    nc.gpsimd.dma_start(
                        y_d[i * 128:(i + 1) * 128, :].rearrange(
                            'p (n t) -> p n t', t=cur_T), src)

    nc._kb_perfetto = getattr(tc_obj, '_perfetto_entries', None)
    nc.compile()
    return nc


def kernel(x, A, params):
    x = np.asarray(x, np.float32)
    dram = prep_weights(A, params)
    # cache key: the sparsity call pattern + shapes (the compiled program
    # depends on which Wbig blocks are nonzero, not on the weight values)
    key = (tuple(
        (b_i, tuple(c.get('gcn_calls', ())), tuple(c.get('res_calls', ())))
        for b_i, c in enumerate(BLOCKS) if c['kind'] == 'conv'),
        tuple(sorted((k, v.shape, str(v.dtype)) for k, v in dram.items())))
    if key not in _CACHE:
        _CACHE.clear()
        _CACHE[key] = build_bass(dram)
    nc = _CACHE[key]

    # x -> per-core [54, NB*T0] layout: row v*3+c, col n*T0+t
    in_maps = []
    for core in range(NCORES):
        xc = x[core * NB:(core + 1) * NB]            # (NB, 3, 12, 18)
        xl = np.ascontiguousarray(xc.transpose(3, 1, 0, 2)).reshape(54, NB * T0)
        m = {'xin': xl.astype(np.float32)}
        m.update(dram)
        in_maps.append(m)

    import time as _time
    trace = bool(int(os.environ.get('KB_TRACE', '0')))
    _t0 = _time.time()
    try:
        res = run_bass_kernel_spmd(nc, in_maps, core_ids=list(range(NCORES)), trace=trace)
    except Exception:
        # transient device wedge: reset cores and retry once
        os.environ['NEURON_RT_RESET_CORES'] = '1'
        _time.sleep(2.0)
        res = run_bass_kernel_spmd(nc, in_maps, core_ids=list(range(NCORES)), trace=trace)
    kernel.last_run_wall_ns = int((_time.time() - _t0) * 1e9)
    if trace and res.exec_time_ns is not None:
        kernel.last_exec_time_ns = res.exec_time_ns
        kernel.last_results = res

    nblocks = int(os.environ.get('KB_NUM_BLOCKS', len(BLOCKS)))
    outs = []
    for core in range(NCORES):
        yc = res.results[core]['y']
        if nblocks == len(BLOCKS):
            # [54, NB*T0] -> (NB, 3, 12, 18)
            out = yc.reshape(18, 3, NB, T0).transpose(2, 1, 3, 0)
            outs.append(out)
        else:
            outs.append(yc)
    if nblocks == len(BLOCKS):
        return np.ascontiguousarray(np.concatenate(outs, axis=0).astype(np.float32))
    return np.stack(outs)


# revision 13
# speedup vs baseline: 1.2380x; 1.2151x over previous
"""ST-GCN autoencoder (nn_GCAE) forward pass on 8 Trainium2 NeuronCores.

Strategy: pure data parallel over batch (64 samples/core). On-chip layout
keeps (joint v, channel c) on SBUF partitions and (sample n, frame t) on the
free dim. The spatial graph conv (gcn) and adjacency multiply are folded into
one block matrix ("Wbig") contracting (c_in, v) -> (c_out, w); the temporal
conv runs as 9 shifted block-diagonal matmuls accumulating in PSUM with
edge-trimmed ranges (no zero padding needed); residual paths accumulate into
the same PSUM via extra matmuls. BN is folded into weights/biases on the
host; PSUM eviction fuses bias+ReLU on the scalar engine.
"""

import os
import numpy as np
import ml_dtypes

import concourse.bacc as bacc
import concourse.mybir as mybir
import concourse.tile as tile
from concourse.bass_utils import run_bass_kernel_spmd

F32 = mybir.dt.float32
F32R = mybir.dt.float32r
BF16 = mybir.dt.bfloat16

K = 3
V = 18
T0 = 12
NBATCH = 512
NCORES = 8
NB = NBATCH // NCORES  # 64 samples per core
EPS_SCALE = 1.0 / np.sqrt(np.float32(1.0 + 1e-5))

ENC_CFG = [(3, 32, 1, False, True), (32, 32, 1, True, True), (32, 32, 2, True, True),
           (32, 48, 1, True, True), (48, 48, 1, True, True), (48, 48, 3, True, True),
           (48, 64, 1, True, True), (64, 64, 1, True, True), (64, 32, 1, True, False)]
DEC_CONV = [(32, 64), (64, 64), (64, 48), (48, 48), (48, 48)]

# jax.image.resize('bilinear') as explicit matrices (half-pixel centers, edge clamp)
def _resize_matrix(t_out, t_in):
    R = np.zeros((t_out, t_in), np.float64)
    for tp in range(t_out):
        src = (tp + 0.5) * t_in / t_out - 0.5
        lo = int(np.floor(src))
        frac = src - lo
        for idx, wgt in ((lo, 1 - frac), (lo + 1, frac)):
            idx = min(max(idx, 0), t_in - 1)
            R[tp, idx] += wgt
    return R

R3 = _resize_matrix(6, 2)
R2 = _resize_matrix(12, 6)


def layout(C, xin=False):
    """Partition layout for channel count C: returns (stride, tiles) where
    tiles = [(v0, nv, P)] and channel c of joint v sits at partition
    (v - v0) * stride + c of its tile."""
    if xin:
        return 3, [(0, 18, 54)]
    s = 64 if C >= 48 else 32
    g = 128 // s
    tiles = []
    v0 = 0
    while v0 < V:
        nv = min(g, V - v0)
        tiles.append((v0, nv, nv * s))
        v0 += nv
    return s, tiles


def build_blocks():
    """Static per-block config (shapes/dtypes only)."""
    blocks = []
    T = T0
    cfgs = []
    for i, (ci, co, st, res, oact) in enumerate(ENC_CFG):
        cfgs.append(dict(kind='conv', src=('enc', i), cin=ci, cout=co, stride=st,
                         residual=res, out_act=oact, out_bn=True))
    cfgs.append(dict(kind='up', scale=3))
    dec_ops = ['conv', 'conv', 'conv', 'up2', 'conv', 'conv']
    ci_dec = 0
    # DEC_OPS = [up3, conv, conv, conv, up2, conv, conv]; up3 already added
    for op in ['conv', 'conv', 'conv', 'up', 'conv', 'conv']:
        if op == 'up':
            cfgs.append(dict(kind='up', scale=2))
        else:
            ci, co = DEC_CONV[ci_dec]
            cfgs.append(dict(kind='conv', src=('dec', ci_dec), cin=ci, cout=co,
                             stride=1, residual=True, out_act=True, out_bn=True))
            ci_dec += 1
    cfgs.append(dict(kind='conv', src=('final', 0), cin=48, cout=3, stride=1,
                     residual=False, out_act=False, out_bn=False))

    for b, c in enumerate(cfgs):
        if c['kind'] == 'up':
            c['T_in'] = T
            T = T * c['scale']
            c['T_out'] = T
        else:
            c['T_in'] = T
            T = T // c['stride']
            c['T_out'] = T
            c['xin'] = (b == 0)
            # residual type
            if c['residual']:
                c['res'] = 'conv' if (c['cin'] != c['cout'] or c['stride'] != 1) else 'id'
            else:
                c['res'] = None
        blocks.append(c)

    # dtype schedule: blocks with free dim < 256 for their matmuls go bf16
    for b, c in enumerate(blocks):
        if c['kind'] != 'conv':
            continue
        c['gcn_dt'] = F32R
        c['res_dt'] = F32R
        c['act_out_dt'] = F32R
    # enc indices in blocks list: 0..8. enc6=idx5 (T_out=2), enc7..9=idx 6,7,8
    for idx in (6, 7, 8):
        blocks[idx]['gcn_dt'] = BF16
        blocks[idx]['res_dt'] = BF16
    for idx in (5, 6, 7, 8):
        blocks[idx]['act_out_dt'] = BF16
    blocks[-1]['act_out_dt'] = F32  # final output, nothing consumes it on PE
    return blocks


BLOCKS = build_blocks()


def chain_layout(nblocks):
    lay = layout(3, xin=True)
    T = T0
    for b in range(nblocks):
        c = BLOCKS[b]
        if c['kind'] == 'conv':
            lay = layout(c['cout'])
            T = c['T_out']
        else:
            T = c['T_out']
    return lay, T


def _fold_cc(Wk, Aef, out_scale, in_lay, out_lay, cin, cout, in_scale=None):
    """Build packed call-blocks for a (c,v)->(c,w) contraction.

    Wk: (Kdim, cout, cin); Aef: (Kdim, V, V); out_scale: (cout,) or None.
    Returns (calls, packed) where calls = [(i, j, col_ofs, ncols)] and
    packed = [128, total_cols] float32; lhsT block for (i,j) is
    packed[0:P_i, col_ofs:col_ofs+P_j].
    """
    s_in, in_tiles = in_lay
    s_out, out_tiles = out_lay
    # F[ci, v, co, w]
    F = np.einsum('koc,kvw->cvow', Wk.astype(np.float64), Aef.astype(np.float64))
    if out_scale is not None:
        F = F * out_scale.astype(np.float64)[None, None, :, None]
    if in_scale is not None:  # in_scale[v, ci]
        F = F * in_scale.astype(np.float64).T[:, :, None, None]
    calls = []
    cols = []
    ofs = 0
    for j, (w0, nw, Pj) in enumerate(out_tiles):
        for i, (v0, nv, Pi) in enumerate(in_tiles):
            M = np.zeros((Pi, Pj), np.float64)
            for vi in range(nv):
                for wj in range(nw):
                    blk = F[:, v0 + vi, :, w0 + wj]  # (cin, cout)
                    M[vi * s_in: vi * s_in + cin, wj * s_out: wj * s_out + cout] = blk
            if np.abs(M).max() == 0.0:
                continue
            calls.append((i, j, ofs, Pj))
            cols.append(M)
            ofs += Pj
    if cols:
        packed = np.zeros((128, ofs), np.float32)
        c0 = 0
        for M in cols:
            packed[0:M.shape[0], c0:c0 + M.shape[1]] = M
            c0 += M.shape[1]
    else:
        packed = np.zeros((128, 0), np.float32)
    return calls, packed


def prep_weights(A, params):
    """Fold BN into weights, build all packed DRAM arrays. Returns dict of
    np arrays plus per-block call metadata (stored into BLOCKS)."""
    A = np.asarray(A, np.float64)
    dram = {}
    g_d = np.asarray(params['data_bn_g'], np.float64) * EPS_SCALE  # (V*C,)
    b_d = np.asarray(params['data_bn_b'], np.float64)
    s_data = g_d.reshape(V, 3)   # [v, ci]
    b_data = b_d.reshape(V, 3)

    for b, c in enumerate(BLOCKS):
        if c['kind'] != 'conv':
            continue
        src, idx = c['src']
        if src == 'enc':
            p = params['enc'][idx]
            ei = np.asarray(params['ei_enc'][idx], np.float64)
        elif src == 'dec':
            p = params['dec'][idx]
            # dec conv blocks map to DEC_OPS positions 1,2,3,5,6
            ei_idx = [1, 2, 3, 5, 6][idx]
            ei = np.asarray(params['ei_dec'][ei_idx], np.float64)
        else:
            p = params['final']
            ei = np.asarray(params['ei_dec'][-1], np.float64)
        Aef = A * ei
        cin, cout = c['cin'], c['cout']
        in_lay_full = layout(cin, xin=c.get('xin', False))
        out_lay_full = layout(cout)
        c['in_lay'] = in_lay_full
        c['out_lay'] = out_lay_full

        Wg = np.asarray(p['gcn_w'], np.float64).reshape(K, cout, cin)
        bg = np.asarray(p['gcn_b'], np.float64).reshape(K, cout)
        s1 = np.asarray(p['bn1_g'], np.float64) * EPS_SCALE
        b1 = np.asarray(p['bn1_b'], np.float64)

        in_scale = s_data if c.get('xin') else None
        calls, packed = _fold_cc(Wg, Aef, s1, in_lay_full, out_lay_full, cin, cout,
                                 in_scale=in_scale)
        c['gcn_calls'] = calls
        np_dt = np.float32 if c['gcn_dt'] != BF16 else ml_dtypes.bfloat16
        dram[f'wbig{b}'] = packed.astype(np_dt)

        # gcn bias: [co, w] = s1*(sum_k bg + data_bn bias fold) + b1
        G = np.einsum('koc,kvw->cvow', Wg, Aef)  # unscaled
        bias_cw = np.einsum('ko,kvw->ow', bg, Aef)
        if c.get('xin'):
            bias_cw = bias_cw + np.einsum('cvow,vc->ow', G, b_data)
        bias_cw = s1[:, None] * bias_cw + b1[:, None]
        s_out, out_tiles = out_lay_full
        bias_arr = np.zeros((128, len(out_tiles)), np.float32)
        for j, (w0, nw, Pj) in enumerate(out_tiles):
            for wj in range(nw):
                bias_arr[wj * s_out: wj * s_out + cout, j] = bias_cw[:, w0 + wj]
        dram[f'bias{b}'] = bias_arr

        # tcn: block-diagonal per-dt weights [128, 9*128]
        Wt = np.asarray(p['tcn_w'], np.float64)[:, :, :, 0]  # (cout, cout, 9)
        bt = np.asarray(p['tcn_b'], np.float64)
        if c['out_bn']:
            s2 = np.asarray(p['bn2_g'], np.float64) * EPS_SCALE
            b2 = np.asarray(p['bn2_b'], np.float64)
        else:
            s2 = np.ones(cout)
            b2 = np.zeros(cout)
        beta = s2 * bt + b2
        wt_arr = np.zeros((128, 9 * 128), np.float64)
        g_out = 128 // s_out
        for dt in range(9):
            for m in range(g_out):
                o = m * s_out
                # lhsT[ci, co] = s2[co] * Wt[co, ci, dt]
                wt_arr[o:o + cout, dt * 128 + o: dt * 128 + o + cout] = \
                    (s2[:, None] * Wt[:, :, dt]).T
        dram[f'wtcn{b}'] = wt_arr.astype(ml_dtypes.bfloat16)

        # residual
        if c['res'] == 'conv':
            Wr = np.asarray(p['res_w'], np.float64)[None]  # (1, cout, cin)
            br = np.asarray(p['res_b'], np.float64)
            sr = np.asarray(p['res_bn_g'], np.float64) * EPS_SCALE
            tr = np.asarray(p['res_bn_b'], np.float64)
            calls_r, packed_r = _fold_cc(Wr, np.eye(V)[None], sr,
                                         in_lay_full, out_lay_full, cin, cout)
            c['res_calls'] = calls_r
            np_dtr = np.float32 if c['res_dt'] != BF16 else ml_dtypes.bfloat16
            dram[f'wres{b}'] = packed_r.astype(np_dtr)
            beta = beta + sr * br + tr
        elif c['res'] == 'id':
            Wr = np.eye(cout)[None]
            calls_r, packed_r = _fold_cc(Wr, np.eye(V)[None], None,
                                         in_lay_full, out_lay_full, cin, cout)
            c['res_calls'] = calls_r
            np_dtr = np.float32 if c['res_dt'] != BF16 else ml_dtypes.bfloat16
            dram[f'wres{b}'] = packed_r.astype(np_dtr)

        beta_arr = np.zeros((128, len(out_tiles)), np.float32)
        for j, (w0, nw, Pj) in enumerate(out_tiles):
            for wj in range(nw):
                beta_arr[wj * s_out: wj * s_out + cout, j] = beta
        dram[f'beta{b}'] = beta_arr
    return dram


_CACHE = {}


def build_bass(dram):
    nc = bacc.Bacc()
    nblocks = int(os.environ.get('KB_NUM_BLOCKS', len(BLOCKS)))
    dram_shapes = {k: (v.shape, v.dtype) for k, v in dram.items()}

    x_d = nc.dram_tensor('xin', [54, NB * T0], F32R, kind='ExternalInput')
    w_d = {}
    for name, arr in dram.items():
        dt = {np.dtype(np.float32): F32, np.dtype(ml_dtypes.bfloat16): BF16}[np.dtype(arr.dtype)]
        if name.startswith('wbig') or name.startswith('wres'):
            if dt == F32:
                dt = F32R
        w_d[name] = nc.dram_tensor(name, list(arr.shape), dt, kind='ExternalInput')

    if nblocks == len(BLOCKS):
        y_d = nc.dram_tensor('y', [54, NB * T0], F32, kind='ExternalOutput')
    else:
        (s_o, t_o), T_fin = chain_layout(nblocks)
        y_d = nc.dram_tensor('y', [len(t_o) * 128, NB * T_fin], F32,
                             kind='ExternalOutput')

    tc_obj = tile.TileContext(nc, trace_sim=bool(int(os.environ.get('KB_SIMTRACE', '0'))))
    with tc_obj as tc:
        with (
            tc.tile_pool(name='consts', bufs=1) as consts,
            tc.tile_pool(name='wpool', bufs=1) as wpool,
            tc.tile_pool(name='acts', bufs=1) as acts,
            tc.tile_pool(name='uptmps', bufs=6) as uptmps,
            tc.tile_pool(name='psum', bufs=4, space='PSUM') as psum,
        ):
            xin = consts.tile([54, NB, T0], F32R)
            nc.gpsimd.dma_start(xin[:], x_d[:].rearrange('p (n t) -> p n t', t=T0))

            cur = [xin]           # list of act tiles
            cur_lay = layout(3, xin=True)
            cur_T = T0

            for b in range(nblocks):
                c = BLOCKS[b]
                if c['kind'] == 'up':
                    R = R3 if c['scale'] == 3 else R2
                    T_in, T_out = c['T_in'], c['T_out']
                    s_l, tiles_l = cur_lay
                    out_tiles = []
                    for i, (v0, nv, Pi) in enumerate(tiles_l):
                        ot = acts.tile([128, NB, T_out], F32R, tag=f'act{b % 2}_{i}', name=f'up{b}_{i}')
                        src = cur[i]
                        if src.dtype == F32R:
                            src = src[:, :, :].bitcast(F32)
                        if c['scale'] == 2:
                            # t'=0 and t'=11 plain copies (on ACT to offload DVE)
                            nc.scalar.copy(ot[0:Pi, :, 0], src[0:Pi, :, 0])
                            nc.scalar.copy(ot[0:Pi, :, T_out - 1], src[0:Pi, :, T_in - 1])
                            # odd outputs 1,3,..,9: 0.75*x[k] + 0.25*x[k+1], k=0..4
                            # even outputs 2,4,..,10: 0.25*x[k] + 0.75*x[k+1]
                            tmp = uptmps.tile([128, NB, T_in - 1], F32, tag='uptmp',
                                              name=f'uptmp{b}_{i}_a')
                            nc.vector.tensor_scalar_mul(
                                tmp[0:Pi], src[0:Pi, :, 1:T_in], 0.25)
                            nc.vector.scalar_tensor_tensor(
                                ot[0:Pi, :, 1:T_out - 1:2], src[0:Pi, :, 0:T_in - 1], 0.75,
                                tmp[0:Pi], mybir.AluOpType.mult, mybir.AluOpType.add)
                            tmp2 = uptmps.tile([128, NB, T_in - 1], F32, tag='uptmp2',
                                               name=f'uptmp{b}_{i}_b')
                            nc.vector.tensor_scalar_mul(
                                tmp2[0:Pi], src[0:Pi, :, 0:T_in - 1], 0.25)
                            nc.vector.scalar_tensor_tensor(
                                ot[0:Pi, :, 2:T_out:2], src[0:Pi, :, 1:T_in], 0.75,
                                tmp2[0:Pi], mybir.AluOpType.mult, mybir.AluOpType.add)
                        else:
                            # scale 3, T 2->6: t'0,1 <- x0; t'4,5 <- x1; t'2,3 blends
                            nc.scalar.copy(ot[0:Pi, :, 0], src[0:Pi, :, 0])
                            nc.scalar.copy(ot[0:Pi, :, 1], src[0:Pi, :, 0])
                            nc.scalar.copy(ot[0:Pi, :, 4], src[0:Pi, :, 1])
                            nc.scalar.copy(ot[0:Pi, :, 5], src[0:Pi, :, 1])
                            tmp = uptmps.tile([128, NB, 2], F32, tag='uptmp',
                                              name=f'uptmp{b}_{i}_a')
                            # columns 2,3: (2/3 x0 + 1/3 x1), (1/3 x0 + 2/3 x1)
                            w23 = float(1.0 / 3.0)
                            nc.vector.tensor_scalar_mul(tmp[0:Pi, :, 0], src[0:Pi, :, 1], w23)
                            nc.vector.tensor_scalar_mul(tmp[0:Pi, :, 1], src[0:Pi, :, 0], w23)
                            nc.vector.scalar_tensor_tensor(
                                ot[0:Pi, :, 2], src[0:Pi, :, 0], float(2.0 / 3.0),
                                tmp[0:Pi, :, 0], mybir.AluOpType.mult, mybir.AluOpType.add)
                            nc.vector.scalar_tensor_tensor(
                                ot[0:Pi, :, 3], src[0:Pi, :, 1], float(2.0 / 3.0),
                                tmp[0:Pi, :, 1], mybir.AluOpType.mult, mybir.AluOpType.add)
                        out_tiles.append(ot)
                    cur = out_tiles
                    cur_T = T_out
                    continue

                # conv block
                T_in, T_out, stride = c['T_in'], c['T_out'], c['stride']
                s_in, in_tiles = c['in_lay']
                s_out, out_tiles_l = c['out_lay']
                n_out = len(out_tiles_l)
                gdt = c['gcn_dt']
                rdt = c['res_dt']
                odt = c['act_out_dt']

                wbig_sh = dram_shapes[f'wbig{b}'][0]
                wbig = wpool.tile([128, max(wbig_sh[1], 1)], gdt, tag=f'wbig{b % 3}')
                if wbig_sh[1] > 0:
                    nc.gpsimd.dma_start(wbig[:, 0:wbig_sh[1]], w_d[f'wbig{b}'][:])
                wtcn = wpool.tile([128, 9 * 128], BF16, tag=f'wtcn{b % 3}')
                nc.gpsimd.dma_start(wtcn[:], w_d[f'wtcn{b}'][:])
                bias_sb = wpool.tile([128, n_out], F32, tag=f'bias{b % 3}')
                nc.gpsimd.dma_start(bias_sb[:], w_d[f'bias{b}'][:])
                beta_sb = wpool.tile([128, n_out], F32, tag=f'beta{b % 3}')
                nc.gpsimd.dma_start(beta_sb[:], w_d[f'beta{b}'][:])
                if c['res'] is not None:
                    wres_sh = dram_shapes[f'wres{b}'][0]
                    wres = wpool.tile([128, max(wres_sh[1], 1)], rdt, tag=f'wres{b % 3}')
                    nc.gpsimd.dma_start(wres[:, 0:wres_sh[1]], w_d[f'wres{b}'][:])

                nchunk = 32 if max(T_in, T_out) >= 12 else NB
                nck = NB // nchunk

                gact = [acts.tile([128, NB, T_in], BF16, tag=f'gact{b % 2}_{i}', name=f'gact{b}_{i}')
                        for i in range(n_out)]
                oact = [acts.tile([128, NB, T_out], odt, tag=f'act{b % 2}_{i}', name=f'oact{b}_{i}')
                        for i in range(n_out)]

                # group gcn calls by output tile
                by_j = {}
                for (i, j, ofs, ncols) in c['gcn_calls']:
                    by_j.setdefault(j, []).append((i, ofs))
                res_by_j = {}
                if c['res'] is not None:
                    for (i, j, ofs, ncols) in c['res_calls']:
                        res_by_j.setdefault(j, []).append((i, ofs))

                for ck in range(nck):
                    n0, n1 = ck * nchunk, (ck + 1) * nchunk
                    # --- gcn (+A, +bn1, +relu) ---
                    for j in range(n_out):
                        w0j, nwj, Pj = out_tiles_l[j]
                        ps = psum.tile([128, nchunk, T_in], F32, tag='ps', bufs=8, name=f'psg{b}_{ck}_{j}')
                        lst = by_j.get(j, [])
                        for q, (i, ofs) in enumerate(lst):
                            v0i, nvi, Pi = in_tiles[i]
                            nc.tensor.matmul(
                                ps[0:Pj, :, :],
                                wbig[0:Pi, ofs:ofs + Pj],
                                cur[i][0:Pi, n0:n1, :],
                                start=(q == 0), stop=(q == len(lst) - 1))
                        if j % 2 == 0:
                            nc.scalar.activation(
                                gact[j][0:Pj, n0:n1, :], ps[0:Pj, :, :],
                                mybir.ActivationFunctionType.Relu,
                                bias=bias_sb[0:Pj, j:j + 1])
                        else:
                            nc.vector.tensor_scalar(
                                gact[j][0:Pj, n0:n1, :], ps[0:Pj, :, :],
                                bias_sb[0:Pj, j:j + 1], 0.0,
                                mybir.AluOpType.add, mybir.AluOpType.max)
                    # --- tcn (+res, +bn2, +beta, +out_act) ---
                    for j in range(n_out):
                        w0j, nwj, Pj = out_tiles_l[j]
                        ps2 = psum.tile([128, nchunk, T_out], F32, tag='ps', bufs=8, name=f'pst{b}_{ck}_{j}')
                        dts = []
                        for dt in range(9):
                            t_lo = max(0, -(-(4 - dt) // stride))
                            # smallest t' with stride*t'+dt-4 >= 0
                            t_lo = max(0, (4 - dt + stride - 1) // stride)
                            # largest t' with stride*t'+dt-4 <= T_in-1
                            t_hi = min(T_out, (T_in - 1 - dt + 4) // stride + 1)
                            if t_hi > t_lo:
                                dts.append((dt, t_lo, t_hi))
                        # center tap first covers full range (start=True)
                        dts.sort(key=lambda z: -(z[2] - z[1]))
                        assert dts[0][1] == 0 and dts[0][2] == T_out, (b, dts)
                        n_calls = len(dts) + len(res_by_j.get(j, []))
                        q = 0
                        for (dt, t_lo, t_hi) in dts:
                            in_lo = stride * t_lo + dt - 4
                            nc.tensor.matmul(
                                ps2[0:Pj, :, t_lo:t_hi],
                                wtcn[0:Pj, dt * 128: dt * 128 + Pj],
                                gact[j][0:Pj, n0:n1, in_lo: in_lo + (t_hi - t_lo - 1) * stride + 1: stride]
                                if stride > 1 else
                                gact[j][0:Pj, n0:n1, in_lo: in_lo + (t_hi - t_lo)],
                                start=(q == 0), stop=(q == n_calls - 1))
                            q += 1
                        for (i, ofs) in res_by_j.get(j, []):
                            v0i, nvi, Pi = in_tiles[i]
                            rhs = cur[i][0:Pi, n0:n1, ::stride] if stride > 1 \
                                else cur[i][0:Pi, n0:n1, :]
                            nc.tensor.matmul(
                                ps2[0:Pj, :, :], wres[0:Pi, ofs:ofs + Pj], rhs,
                                start=False, stop=(q == n_calls - 1))
                            q += 1
                        if j % 2 == 0 or not c['out_act']:
                            func = (mybir.ActivationFunctionType.Relu if c['out_act']
                                    else mybir.ActivationFunctionType.Identity)
                            nc.scalar.activation(
                                oact[j][0:Pj, n0:n1, :], ps2[0:Pj, :, :], func,
                                bias=beta_sb[0:Pj, j:j + 1])
                        else:
                            nc.vector.tensor_scalar(
                                oact[j][0:Pj, n0:n1, :], ps2[0:Pj, :, :],
                                beta_sb[0:Pj, j:j + 1], 0.0,
                                mybir.AluOpType.add, mybir.AluOpType.max)

                cur = oact
                cur_lay = c['out_lay']
                cur_T = T_out

            # write output
            if nblocks == len(BLOCKS):
                s_o, tiles_o = cur_lay
                for v in range(V):
                    ti = None
                    for i, (v0, nv, Pi) in enumerate(tiles_o):
                        if v0 <= v < v0 + nv:
                            ti = i
                            ro = (v - v0) * s_o
                    nc.gpsimd.dma_start(
                        y_d[v * 3:(v + 1) * 3, :].rearrange('p (n t) -> p n t', t=T0),
                        cur[ti][ro:ro + 3, :, :])
            else:
                s_o, tiles_o = cur_lay
                for i, (v0, nv, Pi) in enumerate(tiles_o):
                    src = cur[i][0:128, :, :]
                    if src.dtype == F32R:
                        src = src.bitcast(F32)
                    tmp = acts.tile([128, NB, cur_T], F32, tag=f'dbgout_{i}', name=f'dbg_{i}')
                    nc.vector.tensor_copy(tmp[:], src)
                    src = tmp[:]
                    nc.gpsimd.dma_start(
                        y_d[i * 128:(i + 1) * 128, :].rearrange(
                            'p (n t) -> p n t', t=cur_T), src)

    nc._kb_perfetto = getattr(tc_obj, '_perfetto_entries', None)
    nc.compile()
    return nc


def kernel(x, A, params):
    x = np.asarray(x, np.float32)
    dram = prep_weights(A, params)
    # cache key: the sparsity call pattern + shapes (the compiled program
    # depends on which Wbig blocks are nonzero, not on the weight values)
    key = (tuple(
        (b_i, tuple(c.get('gcn_calls', ())), tuple(c.get('res_calls', ())))
        for b_i, c in enumerate(BLOCKS) if c['kind'] == 'conv'),
        tuple(sorted((k, v.shape, str(v.dtype)) for k, v in dram.items())))
    if key not in _CACHE:
        _CACHE.clear()
        _CACHE[key] = build_bass(dram)
    nc = _CACHE[key]

    # x -> per-core [54, NB*T0] layout: row v*3+c, col n*T0+t
    in_maps = []
    for core in range(NCORES):
        xc = x[core * NB:(core + 1) * NB]            # (NB, 3, 12, 18)
        xl = np.ascontiguousarray(xc.transpose(3, 1, 0, 2)).reshape(54, NB * T0)
        m = {'xin': xl.astype(np.float32)}
        m.update(dram)
        in_maps.append(m)

    import time as _time
    trace = bool(int(os.environ.get('KB_TRACE', '0')))
    _t0 = _time.time()
    try:
        res = run_bass_kernel_spmd(nc, in_maps, core_ids=list(range(NCORES)), trace=trace)
    except Exception:
        # transient device wedge: reset cores and retry once
        os.environ['NEURON_RT_RESET_CORES'] = '1'
        _time.sleep(2.0)
        res = run_bass_kernel_spmd(nc, in_maps, core_ids=list(range(NCORES)), trace=trace)
    kernel.last_run_wall_ns = int((_time.time() - _t0) * 1e9)
    if trace and res.exec_time_ns is not None:
        kernel.last_exec_time_ns = res.exec_time_ns
        kernel.last_results = res

    nblocks = int(os.environ.get('KB_NUM_BLOCKS', len(BLOCKS)))
    outs = []
    for core in range(NCORES):
        yc = res.results[core]['y']
        if nblocks == len(BLOCKS):
            # [54, NB*T0] -> (NB, 3, 12, 18)
            out = yc.reshape(18, 3, NB, T0).transpose(2, 1, 3, 0)
            outs.append(out)
        else:
            outs.append(yc)
    if nblocks == len(BLOCKS):
        return np.ascontiguousarray(np.concatenate(outs, axis=0).astype(np.float32))
    return np.stack(outs)


# revision 14
# speedup vs baseline: 1.3823x; 1.1165x over previous
"""ST-GCN autoencoder (nn_GCAE) forward pass on 8 Trainium2 NeuronCores.

Strategy: pure data parallel over batch (64 samples/core). On-chip layout
keeps (joint v, channel c) on SBUF partitions and (sample n, frame t) on the
free dim. The spatial graph conv (gcn) and adjacency multiply are folded into
one block matrix ("Wbig") contracting (c_in, v) -> (c_out, w); the temporal
conv runs as 9 shifted block-diagonal matmuls accumulating in PSUM with
edge-trimmed ranges (no zero padding needed); residual paths accumulate into
the same PSUM via extra matmuls. BN is folded into weights/biases on the
host; PSUM eviction fuses bias+ReLU on the scalar engine.
"""

import os
import numpy as np
import ml_dtypes

import concourse.bacc as bacc
import concourse.mybir as mybir
import concourse.tile as tile
from concourse.bass_utils import run_bass_kernel_spmd

F32 = mybir.dt.float32
F32R = mybir.dt.float32r
BF16 = mybir.dt.bfloat16

K = 3
V = 18
T0 = 12
NBATCH = 512
NCORES = 8
NB = NBATCH // NCORES  # 64 samples per core
EPS_SCALE = 1.0 / np.sqrt(np.float32(1.0 + 1e-5))

ENC_CFG = [(3, 32, 1, False, True), (32, 32, 1, True, True), (32, 32, 2, True, True),
           (32, 48, 1, True, True), (48, 48, 1, True, True), (48, 48, 3, True, True),
           (48, 64, 1, True, True), (64, 64, 1, True, True), (64, 32, 1, True, False)]
DEC_CONV = [(32, 64), (64, 64), (64, 48), (48, 48), (48, 48)]

# jax.image.resize('bilinear') as explicit matrices (half-pixel centers, edge clamp)
def _resize_matrix(t_out, t_in):
    R = np.zeros((t_out, t_in), np.float64)
    for tp in range(t_out):
        src = (tp + 0.5) * t_in / t_out - 0.5
        lo = int(np.floor(src))
        frac = src - lo
        for idx, wgt in ((lo, 1 - frac), (lo + 1, frac)):
            idx = min(max(idx, 0), t_in - 1)
            R[tp, idx] += wgt
    return R

R3 = _resize_matrix(6, 2)
R2 = _resize_matrix(12, 6)


def layout(C, xin=False):
    """Partition layout for channel count C: returns (stride, tiles) where
    tiles = [(v0, nv, P)] and channel c of joint v sits at partition
    (v - v0) * stride + c of its tile."""
    if xin:
        return 3, [(0, 18, 54)]
    s = 64 if C >= 48 else 32
    g = 128 // s
    tiles = []
    v0 = 0
    while v0 < V:
        nv = min(g, V - v0)
        tiles.append((v0, nv, nv * s))
        v0 += nv
    return s, tiles


def build_blocks():
    """Static per-block config (shapes/dtypes only)."""
    blocks = []
    T = T0
    cfgs = []
    for i, (ci, co, st, res, oact) in enumerate(ENC_CFG):
        cfgs.append(dict(kind='conv', src=('enc', i), cin=ci, cout=co, stride=st,
                         residual=res, out_act=oact, out_bn=True))
    cfgs.append(dict(kind='up', scale=3))
    dec_ops = ['conv', 'conv', 'conv', 'up2', 'conv', 'conv']
    ci_dec = 0
    # DEC_OPS = [up3, conv, conv, conv, up2, conv, conv]; up3 already added
    for op in ['conv', 'conv', 'conv', 'up', 'conv', 'conv']:
        if op == 'up':
            cfgs.append(dict(kind='up', scale=2))
        else:
            ci, co = DEC_CONV[ci_dec]
            cfgs.append(dict(kind='conv', src=('dec', ci_dec), cin=ci, cout=co,
                             stride=1, residual=True, out_act=True, out_bn=True))
            ci_dec += 1
    cfgs.append(dict(kind='conv', src=('final', 0), cin=48, cout=3, stride=1,
                     residual=False, out_act=False, out_bn=False))

    for b, c in enumerate(cfgs):
        if c['kind'] == 'up':
            c['T_in'] = T
            T = T * c['scale']
            c['T_out'] = T
        else:
            c['T_in'] = T
            T = T // c['stride']
            c['T_out'] = T
            c['xin'] = (b == 0)
            # residual type
            if c['residual']:
                c['res'] = 'conv' if (c['cin'] != c['cout'] or c['stride'] != 1) else 'id'
            else:
                c['res'] = None
        blocks.append(c)

    # dtype schedule: blocks with free dim < 256 for their matmuls go bf16
    for b, c in enumerate(blocks):
        if c['kind'] != 'conv':
            continue
        c['gcn_dt'] = F32R
        c['res_dt'] = F32R
        c['act_out_dt'] = F32R
    # enc indices in blocks list: 0..8. enc6=idx5 (T_out=2), enc7..9=idx 6,7,8
    for idx in (6, 7, 8):
        blocks[idx]['gcn_dt'] = BF16
        blocks[idx]['res_dt'] = BF16
    for idx in (5, 6, 7, 8):
        blocks[idx]['act_out_dt'] = BF16
    blocks[-1]['act_out_dt'] = F32  # final output, nothing consumes it on PE
    return blocks


BLOCKS = build_blocks()


def chain_layout(nblocks):
    lay = layout(3, xin=True)
    T = T0
    for b in range(nblocks):
        c = BLOCKS[b]
        if c['kind'] == 'conv':
            lay = layout(c['cout'])
            T = c['T_out']
        else:
            T = c['T_out']
    return lay, T


def _fold_cc(Wk, Aef, out_scale, in_lay, out_lay, cin, cout, in_scale=None):
    """Build packed call-blocks for a (c,v)->(c,w) contraction.

    Wk: (Kdim, cout, cin); Aef: (Kdim, V, V); out_scale: (cout,) or None.
    Returns (calls, packed) where calls = [(i, j, col_ofs, ncols)] and
    packed = [128, total_cols] float32; lhsT block for (i,j) is
    packed[0:P_i, col_ofs:col_ofs+P_j].
    """
    s_in, in_tiles = in_lay
    s_out, out_tiles = out_lay
    # F[ci, v, co, w]
    F = np.einsum('koc,kvw->cvow', Wk.astype(np.float64), Aef.astype(np.float64))
    if out_scale is not None:
        F = F * out_scale.astype(np.float64)[None, None, :, None]
    if in_scale is not None:  # in_scale[v, ci]
        F = F * in_scale.astype(np.float64).T[:, :, None, None]
    calls = []
    cols = []
    ofs = 0
    for j, (w0, nw, Pj) in enumerate(out_tiles):
        for i, (v0, nv, Pi) in enumerate(in_tiles):
            M = np.zeros((Pi, Pj), np.float64)
            for vi in range(nv):
                for wj in range(nw):
                    blk = F[:, v0 + vi, :, w0 + wj]  # (cin, cout)
                    M[vi * s_in: vi * s_in + cin, wj * s_out: wj * s_out + cout] = blk
            if np.abs(M).max() == 0.0:
                continue
            calls.append((i, j, ofs, Pj))
            cols.append(M)
            ofs += Pj
    if cols:
        packed = np.zeros((128, ofs), np.float32)
        c0 = 0
        for M in cols:
            packed[0:M.shape[0], c0:c0 + M.shape[1]] = M
            c0 += M.shape[1]
    else:
        packed = np.zeros((128, 0), np.float32)
    return calls, packed


def prep_weights(A, params):
    """Fold BN into weights, build all packed DRAM arrays. Returns dict of
    np arrays plus per-block call metadata (stored into BLOCKS)."""
    A = np.asarray(A, np.float64)
    dram = {}
    g_d = np.asarray(params['data_bn_g'], np.float64) * EPS_SCALE  # (V*C,)
    b_d = np.asarray(params['data_bn_b'], np.float64)
    s_data = g_d.reshape(V, 3)   # [v, ci]
    b_data = b_d.reshape(V, 3)

    for b, c in enumerate(BLOCKS):
        if c['kind'] != 'conv':
            continue
        src, idx = c['src']
        if src == 'enc':
            p = params['enc'][idx]
            ei = np.asarray(params['ei_enc'][idx], np.float64)
        elif src == 'dec':
            p = params['dec'][idx]
            # dec conv blocks map to DEC_OPS positions 1,2,3,5,6
            ei_idx = [1, 2, 3, 5, 6][idx]
            ei = np.asarray(params['ei_dec'][ei_idx], np.float64)
        else:
            p = params['final']
            ei = np.asarray(params['ei_dec'][-1], np.float64)
        Aef = A * ei
        cin, cout = c['cin'], c['cout']
        in_lay_full = layout(cin, xin=c.get('xin', False))
        out_lay_full = layout(cout)
        c['in_lay'] = in_lay_full
        c['out_lay'] = out_lay_full

        Wg = np.asarray(p['gcn_w'], np.float64).reshape(K, cout, cin)
        bg = np.asarray(p['gcn_b'], np.float64).reshape(K, cout)
        s1 = np.asarray(p['bn1_g'], np.float64) * EPS_SCALE
        b1 = np.asarray(p['bn1_b'], np.float64)

        in_scale = s_data if c.get('xin') else None
        calls, packed = _fold_cc(Wg, Aef, s1, in_lay_full, out_lay_full, cin, cout,
                                 in_scale=in_scale)
        c['gcn_calls'] = calls
        np_dt = np.float32 if c['gcn_dt'] != BF16 else ml_dtypes.bfloat16
        dram[f'wbig{b}'] = packed.astype(np_dt)

        # gcn bias: [co, w] = s1*(sum_k bg + data_bn bias fold) + b1
        G = np.einsum('koc,kvw->cvow', Wg, Aef)  # unscaled
        bias_cw = np.einsum('ko,kvw->ow', bg, Aef)
        if c.get('xin'):
            bias_cw = bias_cw + np.einsum('cvow,vc->ow', G, b_data)
        bias_cw = s1[:, None] * bias_cw + b1[:, None]
        s_out, out_tiles = out_lay_full
        bias_arr = np.zeros((128, len(out_tiles)), np.float32)
        for j, (w0, nw, Pj) in enumerate(out_tiles):
            for wj in range(nw):
                bias_arr[wj * s_out: wj * s_out + cout, j] = bias_cw[:, w0 + wj]
        dram[f'bias{b}'] = bias_arr

        # tcn: block-diagonal per-dt weights [128, 9*128]
        Wt = np.asarray(p['tcn_w'], np.float64)[:, :, :, 0]  # (cout, cout, 9)
        bt = np.asarray(p['tcn_b'], np.float64)
        if c['out_bn']:
            s2 = np.asarray(p['bn2_g'], np.float64) * EPS_SCALE
            b2 = np.asarray(p['bn2_b'], np.float64)
        else:
            s2 = np.ones(cout)
            b2 = np.zeros(cout)
        beta = s2 * bt + b2
        wt_arr = np.zeros((128, 9 * 128), np.float64)
        g_out = 128 // s_out
        for dt in range(9):
            for m in range(g_out):
                o = m * s_out
                # lhsT[ci, co] = s2[co] * Wt[co, ci, dt]
                wt_arr[o:o + cout, dt * 128 + o: dt * 128 + o + cout] = \
                    (s2[:, None] * Wt[:, :, dt]).T
        dram[f'wtcn{b}'] = wt_arr.astype(ml_dtypes.bfloat16)

        # residual
        if c['res'] == 'conv':
            Wr = np.asarray(p['res_w'], np.float64)[None]  # (1, cout, cin)
            br = np.asarray(p['res_b'], np.float64)
            sr = np.asarray(p['res_bn_g'], np.float64) * EPS_SCALE
            tr = np.asarray(p['res_bn_b'], np.float64)
            calls_r, packed_r = _fold_cc(Wr, np.eye(V)[None], sr,
                                         in_lay_full, out_lay_full, cin, cout)
            c['res_calls'] = calls_r
            np_dtr = np.float32 if c['res_dt'] != BF16 else ml_dtypes.bfloat16
            dram[f'wres{b}'] = packed_r.astype(np_dtr)
            beta = beta + sr * br + tr
        elif c['res'] == 'id':
            Wr = np.eye(cout)[None]
            calls_r, packed_r = _fold_cc(Wr, np.eye(V)[None], None,
                                         in_lay_full, out_lay_full, cin, cout)
            c['res_calls'] = calls_r
            np_dtr = np.float32 if c['res_dt'] != BF16 else ml_dtypes.bfloat16
            dram[f'wres{b}'] = packed_r.astype(np_dtr)

        beta_arr = np.zeros((128, len(out_tiles)), np.float32)
        for j, (w0, nw, Pj) in enumerate(out_tiles):
            for wj in range(nw):
                beta_arr[wj * s_out: wj * s_out + cout, j] = beta
        dram[f'beta{b}'] = beta_arr
    return dram


_CACHE = {}


def build_bass(dram):
    nc = bacc.Bacc()
    nblocks = int(os.environ.get('KB_NUM_BLOCKS', len(BLOCKS)))
    dram_shapes = {k: (v.shape, v.dtype) for k, v in dram.items()}

    x_d = nc.dram_tensor('xin', [54, NB * T0], F32R, kind='ExternalInput')
    w_d = {}
    for name, arr in dram.items():
        dt = {np.dtype(np.float32): F32, np.dtype(ml_dtypes.bfloat16): BF16}[np.dtype(arr.dtype)]
        if name.startswith('wbig') or name.startswith('wres'):
            if dt == F32:
                dt = F32R
        w_d[name] = nc.dram_tensor(name, list(arr.shape), dt, kind='ExternalInput')

    if nblocks == len(BLOCKS):
        y_d = nc.dram_tensor('y', [54, NB * T0], F32, kind='ExternalOutput')
    else:
        (s_o, t_o), T_fin = chain_layout(nblocks)
        y_d = nc.dram_tensor('y', [len(t_o) * 128, NB * T_fin], F32,
                             kind='ExternalOutput')

    tc_obj = tile.TileContext(nc, trace_sim=bool(int(os.environ.get('KB_SIMTRACE', '0'))))
    with tc_obj as tc:
        with (
            tc.tile_pool(name='consts', bufs=1) as consts,
            tc.tile_pool(name='wpool', bufs=1) as wpool,
            tc.tile_pool(name='acts', bufs=1) as acts,
            tc.tile_pool(name='uptmps', bufs=6) as uptmps,
            tc.tile_pool(name='psum', bufs=4, space='PSUM') as psum,
        ):
            xin = consts.tile([54, NB, T0], F32R)
            nc.gpsimd.dma_start(xin[:], x_d[:].rearrange('p (n t) -> p n t', t=T0))

            cur = [xin]           # list of act tiles
            cur_lay = layout(3, xin=True)
            cur_T = T0

            for b in range(nblocks):
                c = BLOCKS[b]
                if c['kind'] == 'up':
                    R = R3 if c['scale'] == 3 else R2
                    T_in, T_out = c['T_in'], c['T_out']
                    s_l, tiles_l = cur_lay
                    out_tiles = []
                    for i, (v0, nv, Pi) in enumerate(tiles_l):
                        ot = acts.tile([128, NB, T_out], F32R, tag=f'act{b % 2}_{i}', name=f'up{b}_{i}')
                        src = cur[i]
                        if src.dtype == F32R:
                            src = src[:, :, :].bitcast(F32)
                        if c['scale'] == 2:
                            # t'=0 and t'=11 plain copies (on ACT to offload DVE)
                            nc.scalar.copy(ot[0:Pi, :, 0], src[0:Pi, :, 0])
                            nc.scalar.copy(ot[0:Pi, :, T_out - 1], src[0:Pi, :, T_in - 1])
                            # odd outputs 1,3,..,9: 0.75*x[k] + 0.25*x[k+1], k=0..4
                            # even outputs 2,4,..,10: 0.25*x[k] + 0.75*x[k+1]
                            tmp = uptmps.tile([128, NB, T_in - 1], F32, tag='uptmp',
                                              name=f'uptmp{b}_{i}_a')
                            nc.vector.tensor_scalar_mul(
                                tmp[0:Pi], src[0:Pi, :, 1:T_in], 0.25)
                            nc.vector.scalar_tensor_tensor(
                                ot[0:Pi, :, 1:T_out - 1:2], src[0:Pi, :, 0:T_in - 1], 0.75,
                                tmp[0:Pi], mybir.AluOpType.mult, mybir.AluOpType.add)
                            tmp2 = uptmps.tile([128, NB, T_in - 1], F32, tag='uptmp2',
                                               name=f'uptmp{b}_{i}_b')
                            nc.vector.tensor_scalar_mul(
                                tmp2[0:Pi], src[0:Pi, :, 0:T_in - 1], 0.25)
                            nc.vector.scalar_tensor_tensor(
                                ot[0:Pi, :, 2:T_out:2], src[0:Pi, :, 1:T_in], 0.75,
                                tmp2[0:Pi], mybir.AluOpType.mult, mybir.AluOpType.add)
                        else:
                            # scale 3, T 2->6: t'0,1 <- x0; t'4,5 <- x1; t'2,3 blends
                            nc.scalar.copy(ot[0:Pi, :, 0], src[0:Pi, :, 0])
                            nc.scalar.copy(ot[0:Pi, :, 1], src[0:Pi, :, 0])
                            nc.scalar.copy(ot[0:Pi, :, 4], src[0:Pi, :, 1])
                            nc.scalar.copy(ot[0:Pi, :, 5], src[0:Pi, :, 1])
                            tmp = uptmps.tile([128, NB, 2], F32, tag='uptmp',
                                              name=f'uptmp{b}_{i}_a')
                            # columns 2,3: (2/3 x0 + 1/3 x1), (1/3 x0 + 2/3 x1)
                            w23 = float(1.0 / 3.0)
                            nc.vector.tensor_scalar_mul(tmp[0:Pi, :, 0], src[0:Pi, :, 1], w23)
                            nc.vector.tensor_scalar_mul(tmp[0:Pi, :, 1], src[0:Pi, :, 0], w23)
                            nc.vector.scalar_tensor_tensor(
                                ot[0:Pi, :, 2], src[0:Pi, :, 0], float(2.0 / 3.0),
                                tmp[0:Pi, :, 0], mybir.AluOpType.mult, mybir.AluOpType.add)
                            nc.vector.scalar_tensor_tensor(
                                ot[0:Pi, :, 3], src[0:Pi, :, 1], float(2.0 / 3.0),
                                tmp[0:Pi, :, 1], mybir.AluOpType.mult, mybir.AluOpType.add)
                        out_tiles.append(ot)
                    cur = out_tiles
                    cur_T = T_out
                    continue

                # conv block
                T_in, T_out, stride = c['T_in'], c['T_out'], c['stride']
                s_in, in_tiles = c['in_lay']
                s_out, out_tiles_l = c['out_lay']
                n_out = len(out_tiles_l)
                gdt = c['gcn_dt']
                rdt = c['res_dt']
                odt = c['act_out_dt']

                wbig_sh = dram_shapes[f'wbig{b}'][0]
                wbig = wpool.tile([128, max(wbig_sh[1], 1)], gdt, tag=f'wbig{b % 3}')
                if wbig_sh[1] > 0:
                    nc.gpsimd.dma_start(wbig[:, 0:wbig_sh[1]], w_d[f'wbig{b}'][:])
                wtcn = wpool.tile([128, 9 * 128], BF16, tag=f'wtcn{b % 3}')
                nc.gpsimd.dma_start(wtcn[:], w_d[f'wtcn{b}'][:])
                bias_sb = wpool.tile([128, n_out], F32, tag=f'bias{b % 3}')
                nc.gpsimd.dma_start(bias_sb[:], w_d[f'bias{b}'][:])
                beta_sb = wpool.tile([128, n_out], F32, tag=f'beta{b % 3}')
                nc.gpsimd.dma_start(beta_sb[:], w_d[f'beta{b}'][:])
                if c['res'] is not None:
                    wres_sh = dram_shapes[f'wres{b}'][0]
                    wres = wpool.tile([128, max(wres_sh[1], 1)], rdt, tag=f'wres{b % 3}')
                    nc.gpsimd.dma_start(wres[:, 0:wres_sh[1]], w_d[f'wres{b}'][:])

                nchunk = 32 if max(T_in, T_out) >= 12 else NB
                nck = NB // nchunk

                gact = [acts.tile([128, NB, T_in], BF16, tag=f'gact{b % 2}_{i}', name=f'gact{b}_{i}')
                        for i in range(n_out)]
                oact = [acts.tile([128, NB, T_out], odt, tag=f'act{b % 2}_{i}', name=f'oact{b}_{i}')
                        for i in range(n_out)]

                # group gcn calls by output tile
                by_j = {}
                for (i, j, ofs, ncols) in c['gcn_calls']:
                    by_j.setdefault(j, []).append((i, ofs))
                res_by_j = {}
                if c['res'] is not None:
                    for (i, j, ofs, ncols) in c['res_calls']:
                        res_by_j.setdefault(j, []).append((i, ofs))

                for ck in range(nck):
                    n0, n1 = ck * nchunk, (ck + 1) * nchunk
                    # --- gcn (+A, +bn1, +relu) ---
                    for j in range(n_out):
                        w0j, nwj, Pj = out_tiles_l[j]
                        ps = psum.tile([128, nchunk, T_in], F32, tag='ps', bufs=8, name=f'psg{b}_{ck}_{j}')
                        lst = by_j.get(j, [])
                        for q, (i, ofs) in enumerate(lst):
                            v0i, nvi, Pi = in_tiles[i]
                            nc.tensor.matmul(
                                ps[0:Pj, :, :],
                                wbig[0:Pi, ofs:ofs + Pj],
                                cur[i][0:Pi, n0:n1, :],
                                start=(q == 0), stop=(q == len(lst) - 1))
                        if j % 2 == 0:
                            nc.scalar.activation(
                                gact[j][0:Pj, n0:n1, :], ps[0:Pj, :, :],
                                mybir.ActivationFunctionType.Relu,
                                bias=bias_sb[0:Pj, j:j + 1])
                        else:
                            nc.vector.tensor_scalar(
                                gact[j][0:Pj, n0:n1, :], ps[0:Pj, :, :],
                                bias_sb[0:Pj, j:j + 1], 0.0,
                                mybir.AluOpType.add, mybir.AluOpType.max)
                    # --- tcn (+res, +bn2, +beta, +out_act) ---
                    for j in range(n_out):
                        w0j, nwj, Pj = out_tiles_l[j]
                        ps2 = psum.tile([128, nchunk, T_out], F32, tag='ps', bufs=8, name=f'pst{b}_{ck}_{j}')
                        dts = []
                        for dt in range(9):
                            t_lo = max(0, -(-(4 - dt) // stride))
                            # smallest t' with stride*t'+dt-4 >= 0
                            t_lo = max(0, (4 - dt + stride - 1) // stride)
                            # largest t' with stride*t'+dt-4 <= T_in-1
                            t_hi = min(T_out, (T_in - 1 - dt + 4) // stride + 1)
                            if t_hi > t_lo:
                                dts.append((dt, t_lo, t_hi))
                        # center tap first covers full range (start=True)
                        dts.sort(key=lambda z: -(z[2] - z[1]))
                        assert dts[0][1] == 0 and dts[0][2] == T_out, (b, dts)
                        n_res = len(res_by_j.get(j, [])) if c['res'] == 'conv' else 0
                        n_calls = len(dts) + n_res
                        q = 0
                        for (dt, t_lo, t_hi) in dts:
                            in_lo = stride * t_lo + dt - 4
                            nc.tensor.matmul(
                                ps2[0:Pj, :, t_lo:t_hi],
                                wtcn[0:Pj, dt * 128: dt * 128 + Pj],
                                gact[j][0:Pj, n0:n1, in_lo: in_lo + (t_hi - t_lo - 1) * stride + 1: stride]
                                if stride > 1 else
                                gact[j][0:Pj, n0:n1, in_lo: in_lo + (t_hi - t_lo)],
                                start=(q == 0), stop=(q == n_calls - 1))
                            q += 1
                        if c['res'] != 'id':
                            for (i, ofs) in res_by_j.get(j, []):
                                v0i, nvi, Pi = in_tiles[i]
                                rhs = cur[i][0:Pi, n0:n1, ::stride] if stride > 1 \
                                    else cur[i][0:Pi, n0:n1, :]
                                nc.tensor.matmul(
                                    ps2[0:Pj, :, :], wres[0:Pi, ofs:ofs + Pj], rhs,
                                    start=False, stop=(q == n_calls - 1))
                                q += 1
                        if c['res'] == 'id':
                            xres = cur[j][0:Pj, n0:n1, :]
                            if xres.dtype == F32R:
                                xres = xres.bitcast(F32)
                            nc.vector.scalar_tensor_tensor(
                                oact[j][0:Pj, n0:n1, :], ps2[0:Pj, :, :],
                                beta_sb[0:Pj, j:j + 1], xres,
                                mybir.AluOpType.add, mybir.AluOpType.add)
                            nc.scalar.activation(
                                oact[j][0:Pj, n0:n1, :],
                                oact[j][0:Pj, n0:n1, :] if oact[j].dtype != F32R
                                else oact[j][0:Pj, n0:n1, :].bitcast(F32),
                                mybir.ActivationFunctionType.Relu)
                        elif j % 2 == 0 or not c['out_act']:
                            func = (mybir.ActivationFunctionType.Relu if c['out_act']
                                    else mybir.ActivationFunctionType.Identity)
                            nc.scalar.activation(
                                oact[j][0:Pj, n0:n1, :], ps2[0:Pj, :, :], func,
                                bias=beta_sb[0:Pj, j:j + 1])
                        else:
                            nc.vector.tensor_scalar(
                                oact[j][0:Pj, n0:n1, :], ps2[0:Pj, :, :],
                                beta_sb[0:Pj, j:j + 1], 0.0,
                                mybir.AluOpType.add, mybir.AluOpType.max)

                cur = oact
                cur_lay = c['out_lay']
                cur_T = T_out

            # write output
            if nblocks == len(BLOCKS):
                s_o, tiles_o = cur_lay
                for v in range(V):
                    ti = None
                    for i, (v0, nv, Pi) in enumerate(tiles_o):
                        if v0 <= v < v0 + nv:
                            ti = i
                            ro = (v - v0) * s_o
                    nc.gpsimd.dma_start(
                        y_d[v * 3:(v + 1) * 3, :].rearrange('p (n t) -> p n t', t=T0),
                        cur[ti][ro:ro + 3, :, :])
            else:
                s_o, tiles_o = cur_lay
                for i, (v0, nv, Pi) in enumerate(tiles_o):
                    src = cur[i][0:128, :, :]
                    if src.dtype == F32R:
                        src = src.bitcast(F32)
                    tmp = acts.tile([128, NB, cur_T], F32, tag=f'dbgout_{i}', name=f'dbg_{i}')
                    nc.vector.tensor_copy(tmp[:], src)
                    src = tmp[:]
                    nc.gpsimd.dma_start(
                        y_d[i * 128:(i + 1) * 128, :].rearrange(
                            'p (n t) -> p n t', t=cur_T), src)

    nc._kb_perfetto = getattr(tc_obj, '_perfetto_entries', None)
    nc.compile()
    return nc


def kernel(x, A, params):
    x = np.asarray(x, np.float32)
    dram = prep_weights(A, params)
    # cache key: the sparsity call pattern + shapes (the compiled program
    # depends on which Wbig blocks are nonzero, not on the weight values)
    key = (tuple(
        (b_i, tuple(c.get('gcn_calls', ())), tuple(c.get('res_calls', ())))
        for b_i, c in enumerate(BLOCKS) if c['kind'] == 'conv'),
        tuple(sorted((k, v.shape, str(v.dtype)) for k, v in dram.items())))
    if key not in _CACHE:
        _CACHE.clear()
        _CACHE[key] = build_bass(dram)
    nc = _CACHE[key]

    # x -> per-core [54, NB*T0] layout: row v*3+c, col n*T0+t
    in_maps = []
    for core in range(NCORES):
        xc = x[core * NB:(core + 1) * NB]            # (NB, 3, 12, 18)
        xl = np.ascontiguousarray(xc.transpose(3, 1, 0, 2)).reshape(54, NB * T0)
        m = {'xin': xl.astype(np.float32)}
        m.update(dram)
        in_maps.append(m)

    import time as _time
    trace = bool(int(os.environ.get('KB_TRACE', '0')))
    _t0 = _time.time()
    try:
        res = run_bass_kernel_spmd(nc, in_maps, core_ids=list(range(NCORES)), trace=trace)
    except Exception:
        # transient device wedge: reset cores and retry once
        os.environ['NEURON_RT_RESET_CORES'] = '1'
        _time.sleep(2.0)
        res = run_bass_kernel_spmd(nc, in_maps, core_ids=list(range(NCORES)), trace=trace)
    kernel.last_run_wall_ns = int((_time.time() - _t0) * 1e9)
    if trace and res.exec_time_ns is not None:
        kernel.last_exec_time_ns = res.exec_time_ns
        kernel.last_results = res

    nblocks = int(os.environ.get('KB_NUM_BLOCKS', len(BLOCKS)))
    outs = []
    for core in range(NCORES):
        yc = res.results[core]['y']
        if nblocks == len(BLOCKS):
            # [54, NB*T0] -> (NB, 3, 12, 18)
            out = yc.reshape(18, 3, NB, T0).transpose(2, 1, 3, 0)
            outs.append(out)
        else:
            outs.append(yc)
    if nblocks == len(BLOCKS):
        return np.ascontiguousarray(np.concatenate(outs, axis=0).astype(np.float32))
    return np.stack(outs)


# revision 15
# speedup vs baseline: 1.4597x; 1.0560x over previous
"""ST-GCN autoencoder (nn_GCAE) forward pass on 8 Trainium2 NeuronCores.

Strategy: pure data parallel over batch (64 samples/core). On-chip layout
keeps (joint v, channel c) on SBUF partitions and (sample n, frame t) on the
free dim. The spatial graph conv (gcn) and adjacency multiply are folded into
one block matrix ("Wbig") contracting (c_in, v) -> (c_out, w); the temporal
conv runs as 9 shifted block-diagonal matmuls accumulating in PSUM with
edge-trimmed ranges (no zero padding needed); residual paths accumulate into
the same PSUM via extra matmuls. BN is folded into weights/biases on the
host; PSUM eviction fuses bias+ReLU on the scalar engine.
"""

import os
import numpy as np
import ml_dtypes

import concourse.bacc as bacc
import concourse.mybir as mybir
import concourse.tile as tile
from concourse.bass_utils import run_bass_kernel_spmd

F32 = mybir.dt.float32
F32R = mybir.dt.float32r
BF16 = mybir.dt.bfloat16

K = 3
V = 18
T0 = 12
NBATCH = 512
NCORES = 8
NB = NBATCH // NCORES  # 64 samples per core
EPS_SCALE = 1.0 / np.sqrt(np.float32(1.0 + 1e-5))

ENC_CFG = [(3, 32, 1, False, True), (32, 32, 1, True, True), (32, 32, 2, True, True),
           (32, 48, 1, True, True), (48, 48, 1, True, True), (48, 48, 3, True, True),
           (48, 64, 1, True, True), (64, 64, 1, True, True), (64, 32, 1, True, False)]
DEC_CONV = [(32, 64), (64, 64), (64, 48), (48, 48), (48, 48)]

# jax.image.resize('bilinear') as explicit matrices (half-pixel centers, edge clamp)
def _resize_matrix(t_out, t_in):
    R = np.zeros((t_out, t_in), np.float64)
    for tp in range(t_out):
        src = (tp + 0.5) * t_in / t_out - 0.5
        lo = int(np.floor(src))
        frac = src - lo
        for idx, wgt in ((lo, 1 - frac), (lo + 1, frac)):
            idx = min(max(idx, 0), t_in - 1)
            R[tp, idx] += wgt
    return R

R3 = _resize_matrix(6, 2)
R2 = _resize_matrix(12, 6)


def layout(C, xin=False):
    """Partition layout for channel count C: returns (stride, tiles) where
    tiles = [(v0, nv, P)] and channel c of joint v sits at partition
    (v - v0) * stride + c of its tile."""
    if xin or C == 3:
        return 3, [(0, 18, 54)]
    s = 64 if C >= 48 else 32
    g = 128 // s
    tiles = []
    v0 = 0
    while v0 < V:
        nv = min(g, V - v0)
        tiles.append((v0, nv, nv * s))
        v0 += nv
    return s, tiles


def build_blocks():
    """Static per-block config (shapes/dtypes only)."""
    blocks = []
    T = T0
    cfgs = []
    for i, (ci, co, st, res, oact) in enumerate(ENC_CFG):
        cfgs.append(dict(kind='conv', src=('enc', i), cin=ci, cout=co, stride=st,
                         residual=res, out_act=oact, out_bn=True))
    cfgs.append(dict(kind='up', scale=3))
    dec_ops = ['conv', 'conv', 'conv', 'up2', 'conv', 'conv']
    ci_dec = 0
    # DEC_OPS = [up3, conv, conv, conv, up2, conv, conv]; up3 already added
    for op in ['conv', 'conv', 'conv', 'up', 'conv', 'conv']:
        if op == 'up':
            cfgs.append(dict(kind='up', scale=2))
        else:
            ci, co = DEC_CONV[ci_dec]
            cfgs.append(dict(kind='conv', src=('dec', ci_dec), cin=ci, cout=co,
                             stride=1, residual=True, out_act=True, out_bn=True))
            ci_dec += 1
    cfgs.append(dict(kind='conv', src=('final', 0), cin=48, cout=3, stride=1,
                     residual=False, out_act=False, out_bn=False))

    for b, c in enumerate(cfgs):
        if c['kind'] == 'up':
            c['T_in'] = T
            T = T * c['scale']
            c['T_out'] = T
        else:
            c['T_in'] = T
            T = T // c['stride']
            c['T_out'] = T
            c['xin'] = (b == 0)
            # residual type
            if c['residual']:
                c['res'] = 'conv' if (c['cin'] != c['cout'] or c['stride'] != 1) else 'id'
            else:
                c['res'] = None
        blocks.append(c)

    # dtype schedule: blocks with free dim < 256 for their matmuls go bf16
    for b, c in enumerate(blocks):
        if c['kind'] != 'conv':
            continue
        c['gcn_dt'] = F32R
        c['res_dt'] = F32R
        c['act_out_dt'] = F32R
    # enc indices in blocks list: 0..8. enc6=idx5 (T_out=2), enc7..9=idx 6,7,8
    for idx in (6, 7, 8):
        blocks[idx]['gcn_dt'] = BF16
        blocks[idx]['res_dt'] = BF16
    for idx in (5, 6, 7, 8):
        blocks[idx]['act_out_dt'] = BF16
    blocks[-1]['act_out_dt'] = F32  # final output, nothing consumes it on PE
    return blocks


BLOCKS = build_blocks()


def chain_layout(nblocks):
    lay = layout(3, xin=True)
    T = T0
    for b in range(nblocks):
        c = BLOCKS[b]
        if c['kind'] == 'conv':
            lay = layout(c['cout'])
            T = c['T_out']
        else:
            T = c['T_out']
    return lay, T


def _fold_cc(Wk, Aef, out_scale, in_lay, out_lay, cin, cout, in_scale=None):
    """Build packed call-blocks for a (c,v)->(c,w) contraction.

    Wk: (Kdim, cout, cin); Aef: (Kdim, V, V); out_scale: (cout,) or None.
    Returns (calls, packed) where calls = [(i, j, col_ofs, ncols)] and
    packed = [128, total_cols] float32; lhsT block for (i,j) is
    packed[0:P_i, col_ofs:col_ofs+P_j].
    """
    s_in, in_tiles = in_lay
    s_out, out_tiles = out_lay
    # F[ci, v, co, w]
    F = np.einsum('koc,kvw->cvow', Wk.astype(np.float64), Aef.astype(np.float64))
    if out_scale is not None:
        F = F * out_scale.astype(np.float64)[None, None, :, None]
    if in_scale is not None:  # in_scale[v, ci]
        F = F * in_scale.astype(np.float64).T[:, :, None, None]
    calls = []
    cols = []
    ofs = 0
    for j, (w0, nw, Pj) in enumerate(out_tiles):
        for i, (v0, nv, Pi) in enumerate(in_tiles):
            M = np.zeros((Pi, Pj), np.float64)
            for vi in range(nv):
                for wj in range(nw):
                    blk = F[:, v0 + vi, :, w0 + wj]  # (cin, cout)
                    M[vi * s_in: vi * s_in + cin, wj * s_out: wj * s_out + cout] = blk
            if np.abs(M).max() == 0.0:
                continue
            calls.append((i, j, ofs, Pj))
            cols.append(M)
            ofs += Pj
    if cols:
        packed = np.zeros((128, ofs), np.float32)
        c0 = 0
        for M in cols:
            packed[0:M.shape[0], c0:c0 + M.shape[1]] = M
            c0 += M.shape[1]
    else:
        packed = np.zeros((128, 0), np.float32)
    return calls, packed


def prep_weights(A, params):
    """Fold BN into weights, build all packed DRAM arrays. Returns dict of
    np arrays plus per-block call metadata (stored into BLOCKS)."""
    A = np.asarray(A, np.float64)
    dram = {}
    g_d = np.asarray(params['data_bn_g'], np.float64) * EPS_SCALE  # (V*C,)
    b_d = np.asarray(params['data_bn_b'], np.float64)
    s_data = g_d.reshape(V, 3)   # [v, ci]
    b_data = b_d.reshape(V, 3)

    for b, c in enumerate(BLOCKS):
        if c['kind'] != 'conv':
            continue
        src, idx = c['src']
        if src == 'enc':
            p = params['enc'][idx]
            ei = np.asarray(params['ei_enc'][idx], np.float64)
        elif src == 'dec':
            p = params['dec'][idx]
            # dec conv blocks map to DEC_OPS positions 1,2,3,5,6
            ei_idx = [1, 2, 3, 5, 6][idx]
            ei = np.asarray(params['ei_dec'][ei_idx], np.float64)
        else:
            p = params['final']
            ei = np.asarray(params['ei_dec'][-1], np.float64)
        Aef = A * ei
        cin, cout = c['cin'], c['cout']
        in_lay_full = layout(cin, xin=c.get('xin', False))
        out_lay_full = layout(cout)
        c['in_lay'] = in_lay_full
        c['out_lay'] = out_lay_full

        Wg = np.asarray(p['gcn_w'], np.float64).reshape(K, cout, cin)
        bg = np.asarray(p['gcn_b'], np.float64).reshape(K, cout)
        s1 = np.asarray(p['bn1_g'], np.float64) * EPS_SCALE
        b1 = np.asarray(p['bn1_b'], np.float64)

        in_scale = s_data if c.get('xin') else None
        calls, packed = _fold_cc(Wg, Aef, s1, in_lay_full, out_lay_full, cin, cout,
                                 in_scale=in_scale)
        c['gcn_calls'] = calls
        np_dt = np.float32 if c['gcn_dt'] != BF16 else ml_dtypes.bfloat16
        dram[f'wbig{b}'] = packed.astype(np_dt)

        # gcn bias: [co, w] = s1*(sum_k bg + data_bn bias fold) + b1
        G = np.einsum('koc,kvw->cvow', Wg, Aef)  # unscaled
        bias_cw = np.einsum('ko,kvw->ow', bg, Aef)
        if c.get('xin'):
            bias_cw = bias_cw + np.einsum('cvow,vc->ow', G, b_data)
        bias_cw = s1[:, None] * bias_cw + b1[:, None]
        s_out, out_tiles = out_lay_full
        bias_arr = np.zeros((128, len(out_tiles)), np.float32)
        for j, (w0, nw, Pj) in enumerate(out_tiles):
            for wj in range(nw):
                bias_arr[wj * s_out: wj * s_out + cout, j] = bias_cw[:, w0 + wj]
        dram[f'bias{b}'] = bias_arr

        # tcn: block-diagonal per-dt weights [128, 9*128]
        Wt = np.asarray(p['tcn_w'], np.float64)[:, :, :, 0]  # (cout, cout, 9)
        bt = np.asarray(p['tcn_b'], np.float64)
        if c['out_bn']:
            s2 = np.asarray(p['bn2_g'], np.float64) * EPS_SCALE
            b2 = np.asarray(p['bn2_b'], np.float64)
        else:
            s2 = np.ones(cout)
            b2 = np.zeros(cout)
        beta = s2 * bt + b2
        wt_arr = np.zeros((128, 9 * 128), np.float64)
        g_out = min(128 // s_out, V)
        for dt in range(9):
            for m in range(g_out):
                o = m * s_out
                # lhsT[ci, co] = s2[co] * Wt[co, ci, dt]
                wt_arr[o:o + cout, dt * 128 + o: dt * 128 + o + cout] = \
                    (s2[:, None] * Wt[:, :, dt]).T
        dram[f'wtcn{b}'] = wt_arr.astype(ml_dtypes.bfloat16)

        # residual
        if c['res'] == 'conv':
            Wr = np.asarray(p['res_w'], np.float64)[None]  # (1, cout, cin)
            br = np.asarray(p['res_b'], np.float64)
            sr = np.asarray(p['res_bn_g'], np.float64) * EPS_SCALE
            tr = np.asarray(p['res_bn_b'], np.float64)
            calls_r, packed_r = _fold_cc(Wr, np.eye(V)[None], sr,
                                         in_lay_full, out_lay_full, cin, cout)
            c['res_calls'] = calls_r
            np_dtr = np.float32 if c['res_dt'] != BF16 else ml_dtypes.bfloat16
            dram[f'wres{b}'] = packed_r.astype(np_dtr)
            beta = beta + sr * br + tr
        elif c['res'] == 'id':
            Wr = np.eye(cout)[None]
            calls_r, packed_r = _fold_cc(Wr, np.eye(V)[None], None,
                                         in_lay_full, out_lay_full, cin, cout)
            c['res_calls'] = calls_r
            np_dtr = np.float32 if c['res_dt'] != BF16 else ml_dtypes.bfloat16
            dram[f'wres{b}'] = packed_r.astype(np_dtr)

        beta_arr = np.zeros((128, len(out_tiles)), np.float32)
        for j, (w0, nw, Pj) in enumerate(out_tiles):
            for wj in range(nw):
                beta_arr[wj * s_out: wj * s_out + cout, j] = beta
        dram[f'beta{b}'] = beta_arr
    return dram


_CACHE = {}


def build_bass(dram):
    nc = bacc.Bacc()
    nblocks = int(os.environ.get('KB_NUM_BLOCKS', len(BLOCKS)))
    dram_shapes = {k: (v.shape, v.dtype) for k, v in dram.items()}

    x_d = nc.dram_tensor('xin', [54, NB * T0], F32R, kind='ExternalInput')
    w_d = {}
    for name, arr in dram.items():
        dt = {np.dtype(np.float32): F32, np.dtype(ml_dtypes.bfloat16): BF16}[np.dtype(arr.dtype)]
        if name.startswith('wbig') or name.startswith('wres'):
            if dt == F32:
                dt = F32R
        w_d[name] = nc.dram_tensor(name, list(arr.shape), dt, kind='ExternalInput')

    if nblocks == len(BLOCKS):
        y_d = nc.dram_tensor('y', [54, NB * T0], F32, kind='ExternalOutput')
    else:
        (s_o, t_o), T_fin = chain_layout(nblocks)
        y_d = nc.dram_tensor('y', [len(t_o) * 128, NB * T_fin], F32,
                             kind='ExternalOutput')

    tc_obj = tile.TileContext(nc, trace_sim=bool(int(os.environ.get('KB_SIMTRACE', '0'))))
    with tc_obj as tc:
        with (
            tc.tile_pool(name='consts', bufs=1) as consts,
            tc.tile_pool(name='wpool', bufs=1) as wpool,
            tc.tile_pool(name='acts', bufs=1) as acts,
            tc.tile_pool(name='uptmps', bufs=6) as uptmps,
            tc.tile_pool(name='psum', bufs=4, space='PSUM') as psum,
        ):
            xin = consts.tile([54, NB, T0], F32R)
            nc.gpsimd.dma_start(xin[:], x_d[:].rearrange('p (n t) -> p n t', t=T0))

            cur = [xin]           # list of act tiles
            cur_lay = layout(3, xin=True)
            cur_T = T0

            for b in range(nblocks):
                c = BLOCKS[b]
                if c['kind'] == 'up':
                    R = R3 if c['scale'] == 3 else R2
                    T_in, T_out = c['T_in'], c['T_out']
                    s_l, tiles_l = cur_lay
                    out_tiles = []
                    for i, (v0, nv, Pi) in enumerate(tiles_l):
                        ot = acts.tile([128, NB, T_out], F32R, tag=f'act{b % 2}_{i}', name=f'up{b}_{i}')
                        src = cur[i]
                        if src.dtype == F32R:
                            src = src[:, :, :].bitcast(F32)
                        if c['scale'] == 2:
                            # t'=0 and t'=11 plain copies (on ACT to offload DVE)
                            nc.scalar.copy(ot[0:Pi, :, 0], src[0:Pi, :, 0])
                            nc.scalar.copy(ot[0:Pi, :, T_out - 1], src[0:Pi, :, T_in - 1])
                            # odd outputs 1,3,..,9: 0.75*x[k] + 0.25*x[k+1], k=0..4
                            # even outputs 2,4,..,10: 0.25*x[k] + 0.75*x[k+1]
                            tmp = uptmps.tile([128, NB, T_in - 1], F32, tag='uptmp',
                                              name=f'uptmp{b}_{i}_a')
                            nc.vector.tensor_scalar_mul(
                                tmp[0:Pi], src[0:Pi, :, 1:T_in], 0.25)
                            nc.vector.scalar_tensor_tensor(
                                ot[0:Pi, :, 1:T_out - 1:2], src[0:Pi, :, 0:T_in - 1], 0.75,
                                tmp[0:Pi], mybir.AluOpType.mult, mybir.AluOpType.add)
                            tmp2 = uptmps.tile([128, NB, T_in - 1], F32, tag='uptmp2',
                                               name=f'uptmp{b}_{i}_b')
                            nc.vector.tensor_scalar_mul(
                                tmp2[0:Pi], src[0:Pi, :, 0:T_in - 1], 0.25)
                            nc.vector.scalar_tensor_tensor(
                                ot[0:Pi, :, 2:T_out:2], src[0:Pi, :, 1:T_in], 0.75,
                                tmp2[0:Pi], mybir.AluOpType.mult, mybir.AluOpType.add)
                        else:
                            # scale 3, T 2->6: t'0,1 <- x0; t'4,5 <- x1; t'2,3 blends
                            nc.scalar.copy(ot[0:Pi, :, 0], src[0:Pi, :, 0])
                            nc.scalar.copy(ot[0:Pi, :, 1], src[0:Pi, :, 0])
                            nc.scalar.copy(ot[0:Pi, :, 4], src[0:Pi, :, 1])
                            nc.scalar.copy(ot[0:Pi, :, 5], src[0:Pi, :, 1])
                            tmp = uptmps.tile([128, NB, 2], F32, tag='uptmp',
                                              name=f'uptmp{b}_{i}_a')
                            # columns 2,3: (2/3 x0 + 1/3 x1), (1/3 x0 + 2/3 x1)
                            w23 = float(1.0 / 3.0)
                            nc.vector.tensor_scalar_mul(tmp[0:Pi, :, 0], src[0:Pi, :, 1], w23)
                            nc.vector.tensor_scalar_mul(tmp[0:Pi, :, 1], src[0:Pi, :, 0], w23)
                            nc.vector.scalar_tensor_tensor(
                                ot[0:Pi, :, 2], src[0:Pi, :, 0], float(2.0 / 3.0),
                                tmp[0:Pi, :, 0], mybir.AluOpType.mult, mybir.AluOpType.add)
                            nc.vector.scalar_tensor_tensor(
                                ot[0:Pi, :, 3], src[0:Pi, :, 1], float(2.0 / 3.0),
                                tmp[0:Pi, :, 1], mybir.AluOpType.mult, mybir.AluOpType.add)
                        out_tiles.append(ot)
                    cur = out_tiles
                    cur_T = T_out
                    continue

                # conv block
                T_in, T_out, stride = c['T_in'], c['T_out'], c['stride']
                s_in, in_tiles = c['in_lay']
                s_out, out_tiles_l = c['out_lay']
                n_out = len(out_tiles_l)
                gdt = c['gcn_dt']
                rdt = c['res_dt']
                odt = c['act_out_dt']

                wbig_sh = dram_shapes[f'wbig{b}'][0]
                wbig = wpool.tile([128, max(wbig_sh[1], 1)], gdt, tag=f'wbig{b % 3}')
                if wbig_sh[1] > 0:
                    nc.gpsimd.dma_start(wbig[:, 0:wbig_sh[1]], w_d[f'wbig{b}'][:])
                wtcn = wpool.tile([128, 9 * 128], BF16, tag=f'wtcn{b % 3}')
                nc.gpsimd.dma_start(wtcn[:], w_d[f'wtcn{b}'][:])
                bias_sb = wpool.tile([128, n_out], F32, tag=f'bias{b % 3}')
                nc.gpsimd.dma_start(bias_sb[:], w_d[f'bias{b}'][:])
                beta_sb = wpool.tile([128, n_out], F32, tag=f'beta{b % 3}')
                nc.gpsimd.dma_start(beta_sb[:], w_d[f'beta{b}'][:])
                if c['res'] is not None:
                    wres_sh = dram_shapes[f'wres{b}'][0]
                    wres = wpool.tile([128, max(wres_sh[1], 1)], rdt, tag=f'wres{b % 3}')
                    nc.gpsimd.dma_start(wres[:, 0:wres_sh[1]], w_d[f'wres{b}'][:])

                nchunk = 32 if max(T_in, T_out) >= 12 else NB
                nck = NB // nchunk

                gact = [acts.tile([128, NB, T_in], BF16, tag=f'gact{b % 2}_{i}', name=f'gact{b}_{i}')
                        for i in range(n_out)]
                oact = [acts.tile([128, NB, T_out], odt, tag=f'act{b % 2}_{i}', name=f'oact{b}_{i}')
                        for i in range(n_out)]

                # group gcn calls by output tile
                by_j = {}
                for (i, j, ofs, ncols) in c['gcn_calls']:
                    by_j.setdefault(j, []).append((i, ofs))
                res_by_j = {}
                if c['res'] is not None:
                    for (i, j, ofs, ncols) in c['res_calls']:
                        res_by_j.setdefault(j, []).append((i, ofs))

                for ck in range(nck):
                    n0, n1 = ck * nchunk, (ck + 1) * nchunk
                    # --- gcn (+A, +bn1, +relu) ---
                    for j in range(n_out):
                        w0j, nwj, Pj = out_tiles_l[j]
                        ps = psum.tile([128, nchunk, T_in], F32, tag='ps', bufs=8, name=f'psg{b}_{ck}_{j}')
                        lst = by_j.get(j, [])
                        for q, (i, ofs) in enumerate(lst):
                            v0i, nvi, Pi = in_tiles[i]
                            nc.tensor.matmul(
                                ps[0:Pj, :, :],
                                wbig[0:Pi, ofs:ofs + Pj],
                                cur[i][0:Pi, n0:n1, :],
                                start=(q == 0), stop=(q == len(lst) - 1))
                        if j % 2 == 0:
                            nc.scalar.activation(
                                gact[j][0:Pj, n0:n1, :], ps[0:Pj, :, :],
                                mybir.ActivationFunctionType.Relu,
                                bias=bias_sb[0:Pj, j:j + 1])
                        else:
                            nc.vector.tensor_scalar(
                                gact[j][0:Pj, n0:n1, :], ps[0:Pj, :, :],
                                bias_sb[0:Pj, j:j + 1], 0.0,
                                mybir.AluOpType.add, mybir.AluOpType.max)
                    # --- tcn (+res, +bn2, +beta, +out_act) ---
                    for j in range(n_out):
                        w0j, nwj, Pj = out_tiles_l[j]
                        ps2 = psum.tile([128, nchunk, T_out], F32, tag='ps', bufs=8, name=f'pst{b}_{ck}_{j}')
                        dts = []
                        for dt in range(9):
                            t_lo = max(0, -(-(4 - dt) // stride))
                            # smallest t' with stride*t'+dt-4 >= 0
                            t_lo = max(0, (4 - dt + stride - 1) // stride)
                            # largest t' with stride*t'+dt-4 <= T_in-1
                            t_hi = min(T_out, (T_in - 1 - dt + 4) // stride + 1)
                            if t_hi > t_lo:
                                dts.append((dt, t_lo, t_hi))
                        # center tap first covers full range (start=True)
                        dts.sort(key=lambda z: -(z[2] - z[1]))
                        assert dts[0][1] == 0 and dts[0][2] == T_out, (b, dts)
                        n_res = len(res_by_j.get(j, [])) if c['res'] == 'conv' else 0
                        n_calls = len(dts) + n_res
                        q = 0
                        for (dt, t_lo, t_hi) in dts:
                            in_lo = stride * t_lo + dt - 4
                            nc.tensor.matmul(
                                ps2[0:Pj, :, t_lo:t_hi],
                                wtcn[0:Pj, dt * 128: dt * 128 + Pj],
                                gact[j][0:Pj, n0:n1, in_lo: in_lo + (t_hi - t_lo - 1) * stride + 1: stride]
                                if stride > 1 else
                                gact[j][0:Pj, n0:n1, in_lo: in_lo + (t_hi - t_lo)],
                                start=(q == 0), stop=(q == n_calls - 1))
                            q += 1
                        if c['res'] != 'id':
                            for (i, ofs) in res_by_j.get(j, []):
                                v0i, nvi, Pi = in_tiles[i]
                                rhs = cur[i][0:Pi, n0:n1, ::stride] if stride > 1 \
                                    else cur[i][0:Pi, n0:n1, :]
                                nc.tensor.matmul(
                                    ps2[0:Pj, :, :], wres[0:Pi, ofs:ofs + Pj], rhs,
                                    start=False, stop=(q == n_calls - 1))
                                q += 1
                        if c['res'] == 'id':
                            xres = cur[j][0:Pj, n0:n1, :]
                            if xres.dtype == F32R:
                                xres = xres.bitcast(F32)
                            nc.vector.scalar_tensor_tensor(
                                oact[j][0:Pj, n0:n1, :], ps2[0:Pj, :, :],
                                beta_sb[0:Pj, j:j + 1], xres,
                                mybir.AluOpType.add, mybir.AluOpType.add)
                            nc.scalar.activation(
                                oact[j][0:Pj, n0:n1, :],
                                oact[j][0:Pj, n0:n1, :] if oact[j].dtype != F32R
                                else oact[j][0:Pj, n0:n1, :].bitcast(F32),
                                mybir.ActivationFunctionType.Relu)
                        elif j % 2 == 0 or not c['out_act']:
                            func = (mybir.ActivationFunctionType.Relu if c['out_act']
                                    else mybir.ActivationFunctionType.Identity)
                            nc.scalar.activation(
                                oact[j][0:Pj, n0:n1, :], ps2[0:Pj, :, :], func,
                                bias=beta_sb[0:Pj, j:j + 1])
                        else:
                            nc.vector.tensor_scalar(
                                oact[j][0:Pj, n0:n1, :], ps2[0:Pj, :, :],
                                beta_sb[0:Pj, j:j + 1], 0.0,
                                mybir.AluOpType.add, mybir.AluOpType.max)

                cur = oact
                cur_lay = c['out_lay']
                cur_T = T_out

            # write output
            if nblocks == len(BLOCKS):
                s_o, tiles_o = cur_lay
                if len(tiles_o) == 1 and s_o == 3:
                    nc.gpsimd.dma_start(
                        y_d[:, :].rearrange('p (n t) -> p n t', t=T0),
                        cur[0][0:54, :, :])
                else:
                    for v in range(V):
                        ti = None
                        for i, (v0, nv, Pi) in enumerate(tiles_o):
                            if v0 <= v < v0 + nv:
                                ti = i
                                ro = (v - v0) * s_o
                        nc.gpsimd.dma_start(
                            y_d[v * 3:(v + 1) * 3, :].rearrange('p (n t) -> p n t', t=T0),
                            cur[ti][ro:ro + 3, :, :])
            else:
                s_o, tiles_o = cur_lay
                for i, (v0, nv, Pi) in enumerate(tiles_o):
                    src = cur[i][0:128, :, :]
                    if src.dtype == F32R:
                        src = src.bitcast(F32)
                    tmp = acts.tile([128, NB, cur_T], F32, tag=f'dbgout_{i}', name=f'dbg_{i}')
                    nc.vector.tensor_copy(tmp[:], src)
                    src = tmp[:]
                    nc.gpsimd.dma_start(
                        y_d[i * 128:(i + 1) * 128, :].rearrange(
                            'p (n t) -> p n t', t=cur_T), src)

    nc._kb_perfetto = getattr(tc_obj, '_perfetto_entries', None)
    nc.compile()
    return nc


def kernel(x, A, params):
    x = np.asarray(x, np.float32)
    dram = prep_weights(A, params)
    # cache key: the sparsity call pattern + shapes (the compiled program
    # depends on which Wbig blocks are nonzero, not on the weight values)
    key = (tuple(
        (b_i, tuple(c.get('gcn_calls', ())), tuple(c.get('res_calls', ())))
        for b_i, c in enumerate(BLOCKS) if c['kind'] == 'conv'),
        tuple(sorted((k, v.shape, str(v.dtype)) for k, v in dram.items())))
    if key not in _CACHE:
        _CACHE.clear()
        _CACHE[key] = build_bass(dram)
    nc = _CACHE[key]

    # x -> per-core [54, NB*T0] layout: row v*3+c, col n*T0+t
    in_maps = []
    for core in range(NCORES):
        xc = x[core * NB:(core + 1) * NB]            # (NB, 3, 12, 18)
        xl = np.ascontiguousarray(xc.transpose(3, 1, 0, 2)).reshape(54, NB * T0)
        m = {'xin': xl.astype(np.float32)}
        m.update(dram)
        in_maps.append(m)

    import time as _time
    trace = bool(int(os.environ.get('KB_TRACE', '0')))
    _t0 = _time.time()
    try:
        res = run_bass_kernel_spmd(nc, in_maps, core_ids=list(range(NCORES)), trace=trace)
    except Exception:
        # transient device wedge: reset cores and retry once
        os.environ['NEURON_RT_RESET_CORES'] = '1'
        _time.sleep(2.0)
        res = run_bass_kernel_spmd(nc, in_maps, core_ids=list(range(NCORES)), trace=trace)
    kernel.last_run_wall_ns = int((_time.time() - _t0) * 1e9)
    if trace and res.exec_time_ns is not None:
        kernel.last_exec_time_ns = res.exec_time_ns
        kernel.last_results = res

    nblocks = int(os.environ.get('KB_NUM_BLOCKS', len(BLOCKS)))
    outs = []
    for core in range(NCORES):
        yc = res.results[core]['y']
        if nblocks == len(BLOCKS):
            # [54, NB*T0] -> (NB, 3, 12, 18)
            out = yc.reshape(18, 3, NB, T0).transpose(2, 1, 3, 0)
            outs.append(out)
        else:
            outs.append(yc)
    if nblocks == len(BLOCKS):
        return np.ascontiguousarray(np.concatenate(outs, axis=0).astype(np.float32))
    return np.stack(outs)


# revision 16
# speedup vs baseline: 1.4847x; 1.0171x over previous
"""ST-GCN autoencoder (nn_GCAE) forward pass on 8 Trainium2 NeuronCores.

Strategy: pure data parallel over batch (64 samples/core). On-chip layout
keeps (joint v, channel c) on SBUF partitions and (sample n, frame t) on the
free dim. The spatial graph conv (gcn) and adjacency multiply are folded into
one block matrix ("Wbig") contracting (c_in, v) -> (c_out, w); the temporal
conv runs as 9 shifted block-diagonal matmuls accumulating in PSUM with
edge-trimmed ranges (no zero padding needed); residual paths accumulate into
the same PSUM via extra matmuls. BN is folded into weights/biases on the
host; PSUM eviction fuses bias+ReLU on the scalar engine.
"""

import os
import numpy as np
import ml_dtypes

import concourse.bacc as bacc
import concourse.mybir as mybir
import concourse.tile as tile
from concourse.bass_utils import run_bass_kernel_spmd

F32 = mybir.dt.float32
F32R = mybir.dt.float32r
BF16 = mybir.dt.bfloat16

K = 3
V = 18
T0 = 12
NBATCH = 512
NCORES = 8
NB = NBATCH // NCORES  # 64 samples per core
EPS_SCALE = 1.0 / np.sqrt(np.float32(1.0 + 1e-5))

ENC_CFG = [(3, 32, 1, False, True), (32, 32, 1, True, True), (32, 32, 2, True, True),
           (32, 48, 1, True, True), (48, 48, 1, True, True), (48, 48, 3, True, True),
           (48, 64, 1, True, True), (64, 64, 1, True, True), (64, 32, 1, True, False)]
DEC_CONV = [(32, 64), (64, 64), (64, 48), (48, 48), (48, 48)]

# jax.image.resize('bilinear') as explicit matrices (half-pixel centers, edge clamp)
def _resize_matrix(t_out, t_in):
    R = np.zeros((t_out, t_in), np.float64)
    for tp in range(t_out):
        src = (tp + 0.5) * t_in / t_out - 0.5
        lo = int(np.floor(src))
        frac = src - lo
        for idx, wgt in ((lo, 1 - frac), (lo + 1, frac)):
            idx = min(max(idx, 0), t_in - 1)
            R[tp, idx] += wgt
    return R

R3 = _resize_matrix(6, 2)
R2 = _resize_matrix(12, 6)


def layout(C, xin=False):
    """Partition layout for channel count C: returns (stride, tiles) where
    tiles = [(v0, nv, P)] and channel c of joint v sits at partition
    (v - v0) * stride + c of its tile."""
    if xin or C == 3:
        return 3, [(0, 18, 54)]
    s = 64 if C >= 48 else 32
    g = 128 // s
    tiles = []
    v0 = 0
    while v0 < V:
        nv = min(g, V - v0)
        tiles.append((v0, nv, nv * s))
        v0 += nv
    return s, tiles


def build_blocks():
    """Static per-block config (shapes/dtypes only)."""
    blocks = []
    T = T0
    cfgs = []
    for i, (ci, co, st, res, oact) in enumerate(ENC_CFG):
        cfgs.append(dict(kind='conv', src=('enc', i), cin=ci, cout=co, stride=st,
                         residual=res, out_act=oact, out_bn=True))
    cfgs.append(dict(kind='up', scale=3))
    dec_ops = ['conv', 'conv', 'conv', 'up2', 'conv', 'conv']
    ci_dec = 0
    # DEC_OPS = [up3, conv, conv, conv, up2, conv, conv]; up3 already added
    for op in ['conv', 'conv', 'conv', 'up', 'conv', 'conv']:
        if op == 'up':
            cfgs.append(dict(kind='up', scale=2))
        else:
            ci, co = DEC_CONV[ci_dec]
            cfgs.append(dict(kind='conv', src=('dec', ci_dec), cin=ci, cout=co,
                             stride=1, residual=True, out_act=True, out_bn=True))
            ci_dec += 1
    cfgs.append(dict(kind='conv', src=('final', 0), cin=48, cout=3, stride=1,
                     residual=False, out_act=False, out_bn=False))

    for b, c in enumerate(cfgs):
        if c['kind'] == 'up':
            c['T_in'] = T
            T = T * c['scale']
            c['T_out'] = T
        else:
            c['T_in'] = T
            T = T // c['stride']
            c['T_out'] = T
            c['xin'] = (b == 0)
            # residual type
            if c['residual']:
                c['res'] = 'conv' if (c['cin'] != c['cout'] or c['stride'] != 1) else 'id'
            else:
                c['res'] = None
        blocks.append(c)

    # dtype schedule: blocks with free dim < 256 for their matmuls go bf16
    for b, c in enumerate(blocks):
        if c['kind'] != 'conv':
            continue
        c['gcn_dt'] = F32R
        c['res_dt'] = F32R
        c['act_out_dt'] = F32R
    # enc indices in blocks list: 0..8. enc6=idx5 (T_out=2), enc7..9=idx 6,7,8
    for idx in (6, 7, 8):
        blocks[idx]['gcn_dt'] = BF16
        blocks[idx]['res_dt'] = BF16
    for idx in (5, 6, 7, 8):
        blocks[idx]['act_out_dt'] = BF16
    blocks[-1]['act_out_dt'] = F32  # final output, nothing consumes it on PE
    return blocks


BLOCKS = build_blocks()


def chain_layout(nblocks):
    lay = layout(3, xin=True)
    T = T0
    for b in range(nblocks):
        c = BLOCKS[b]
        if c['kind'] == 'conv':
            lay = layout(c['cout'])
            T = c['T_out']
        else:
            T = c['T_out']
    return lay, T


def _fold_cc(Wk, Aef, out_scale, in_lay, out_lay, cin, cout, in_scale=None):
    """Build packed call-blocks for a (c,v)->(c,w) contraction.

    Wk: (Kdim, cout, cin); Aef: (Kdim, V, V); out_scale: (cout,) or None.
    Returns (calls, packed) where calls = [(i, j, col_ofs, ncols)] and
    packed = [128, total_cols] float32; lhsT block for (i,j) is
    packed[0:P_i, col_ofs:col_ofs+P_j].
    """
    s_in, in_tiles = in_lay
    s_out, out_tiles = out_lay
    # F[ci, v, co, w]
    F = np.einsum('koc,kvw->cvow', Wk.astype(np.float64), Aef.astype(np.float64))
    if out_scale is not None:
        F = F * out_scale.astype(np.float64)[None, None, :, None]
    if in_scale is not None:  # in_scale[v, ci]
        F = F * in_scale.astype(np.float64).T[:, :, None, None]
    calls = []
    cols = []
    ofs = 0
    for j, (w0, nw, Pj) in enumerate(out_tiles):
        for i, (v0, nv, Pi) in enumerate(in_tiles):
            M = np.zeros((Pi, Pj), np.float64)
            for vi in range(nv):
                for wj in range(nw):
                    blk = F[:, v0 + vi, :, w0 + wj]  # (cin, cout)
                    M[vi * s_in: vi * s_in + cin, wj * s_out: wj * s_out + cout] = blk
            if np.abs(M).max() == 0.0:
                continue
            calls.append((i, j, ofs, Pj))
            cols.append(M)
            ofs += Pj
    if cols:
        packed = np.zeros((128, ofs), np.float32)
        c0 = 0
        for M in cols:
            packed[0:M.shape[0], c0:c0 + M.shape[1]] = M
            c0 += M.shape[1]
    else:
        packed = np.zeros((128, 0), np.float32)
    return calls, packed


def prep_weights(A, params):
    """Fold BN into weights, build all packed DRAM arrays. Returns dict of
    np arrays plus per-block call metadata (stored into BLOCKS)."""
    A = np.asarray(A, np.float64)
    dram = {}
    g_d = np.asarray(params['data_bn_g'], np.float64) * EPS_SCALE  # (V*C,)
    b_d = np.asarray(params['data_bn_b'], np.float64)
    s_data = g_d.reshape(V, 3)   # [v, ci]
    b_data = b_d.reshape(V, 3)

    for b, c in enumerate(BLOCKS):
        if c['kind'] != 'conv':
            continue
        src, idx = c['src']
        if src == 'enc':
            p = params['enc'][idx]
            ei = np.asarray(params['ei_enc'][idx], np.float64)
        elif src == 'dec':
            p = params['dec'][idx]
            # dec conv blocks map to DEC_OPS positions 1,2,3,5,6
            ei_idx = [1, 2, 3, 5, 6][idx]
            ei = np.asarray(params['ei_dec'][ei_idx], np.float64)
        else:
            p = params['final']
            ei = np.asarray(params['ei_dec'][-1], np.float64)
        Aef = A * ei
        cin, cout = c['cin'], c['cout']
        in_lay_full = layout(cin, xin=c.get('xin', False))
        out_lay_full = layout(cout)
        c['in_lay'] = in_lay_full
        c['out_lay'] = out_lay_full

        Wg = np.asarray(p['gcn_w'], np.float64).reshape(K, cout, cin)
        bg = np.asarray(p['gcn_b'], np.float64).reshape(K, cout)
        s1 = np.asarray(p['bn1_g'], np.float64) * EPS_SCALE
        b1 = np.asarray(p['bn1_b'], np.float64)

        in_scale = s_data if c.get('xin') else None
        calls, packed = _fold_cc(Wg, Aef, s1, in_lay_full, out_lay_full, cin, cout,
                                 in_scale=in_scale)
        c['gcn_calls'] = calls
        np_dt = np.float32 if c['gcn_dt'] != BF16 else ml_dtypes.bfloat16
        dram[f'wbig{b}'] = packed.astype(np_dt)

        # gcn bias: [co, w] = s1*(sum_k bg + data_bn bias fold) + b1
        G = np.einsum('koc,kvw->cvow', Wg, Aef)  # unscaled
        bias_cw = np.einsum('ko,kvw->ow', bg, Aef)
        if c.get('xin'):
            bias_cw = bias_cw + np.einsum('cvow,vc->ow', G, b_data)
        bias_cw = s1[:, None] * bias_cw + b1[:, None]
        s_out, out_tiles = out_lay_full
        bias_arr = np.zeros((128, len(out_tiles)), np.float32)
        for j, (w0, nw, Pj) in enumerate(out_tiles):
            for wj in range(nw):
                bias_arr[wj * s_out: wj * s_out + cout, j] = bias_cw[:, w0 + wj]
        dram[f'bias{b}'] = bias_arr

        # tcn: block-diagonal per-dt weights [128, 9*128]
        Wt = np.asarray(p['tcn_w'], np.float64)[:, :, :, 0]  # (cout, cout, 9)
        bt = np.asarray(p['tcn_b'], np.float64)
        if c['out_bn']:
            s2 = np.asarray(p['bn2_g'], np.float64) * EPS_SCALE
            b2 = np.asarray(p['bn2_b'], np.float64)
        else:
            s2 = np.ones(cout)
            b2 = np.zeros(cout)
        beta = s2 * bt + b2
        wt_arr = np.zeros((128, 9 * 128), np.float64)
        g_out = min(128 // s_out, V)
        for dt in range(9):
            for m in range(g_out):
                o = m * s_out
                # lhsT[ci, co] = s2[co] * Wt[co, ci, dt]
                wt_arr[o:o + cout, dt * 128 + o: dt * 128 + o + cout] = \
                    (s2[:, None] * Wt[:, :, dt]).T
        dram[f'wtcn{b}'] = wt_arr.astype(ml_dtypes.bfloat16)

        # residual
        if c['res'] == 'conv':
            Wr = np.asarray(p['res_w'], np.float64)[None]  # (1, cout, cin)
            br = np.asarray(p['res_b'], np.float64)
            sr = np.asarray(p['res_bn_g'], np.float64) * EPS_SCALE
            tr = np.asarray(p['res_bn_b'], np.float64)
            calls_r, packed_r = _fold_cc(Wr, np.eye(V)[None], sr,
                                         in_lay_full, out_lay_full, cin, cout)
            c['res_calls'] = calls_r
            np_dtr = np.float32 if c['res_dt'] != BF16 else ml_dtypes.bfloat16
            dram[f'wres{b}'] = packed_r.astype(np_dtr)
            beta = beta + sr * br + tr
        elif c['res'] == 'id':
            Wr = np.eye(cout)[None]
            calls_r, packed_r = _fold_cc(Wr, np.eye(V)[None], None,
                                         in_lay_full, out_lay_full, cin, cout)
            c['res_calls'] = calls_r
            np_dtr = np.float32 if c['res_dt'] != BF16 else ml_dtypes.bfloat16
            dram[f'wres{b}'] = packed_r.astype(np_dtr)

        beta_arr = np.zeros((128, len(out_tiles)), np.float32)
        for j, (w0, nw, Pj) in enumerate(out_tiles):
            for wj in range(nw):
                beta_arr[wj * s_out: wj * s_out + cout, j] = beta
        dram[f'beta{b}'] = beta_arr
    return dram


_CACHE = {}


def build_bass(dram):
    nc = bacc.Bacc()
    nblocks = int(os.environ.get('KB_NUM_BLOCKS', len(BLOCKS)))
    dram_shapes = {k: (v.shape, v.dtype) for k, v in dram.items()}

    x_d = nc.dram_tensor('xin', [54, NB * T0], F32R, kind='ExternalInput')
    w_d = {}
    for name, arr in dram.items():
        dt = {np.dtype(np.float32): F32, np.dtype(ml_dtypes.bfloat16): BF16}[np.dtype(arr.dtype)]
        if name.startswith('wbig') or name.startswith('wres'):
            if dt == F32:
                dt = F32R
        w_d[name] = nc.dram_tensor(name, list(arr.shape), dt, kind='ExternalInput')

    if nblocks == len(BLOCKS):
        y_d = nc.dram_tensor('y', [54, NB * T0], F32, kind='ExternalOutput')
    else:
        (s_o, t_o), T_fin = chain_layout(nblocks)
        y_d = nc.dram_tensor('y', [len(t_o) * 128, NB * T_fin], F32,
                             kind='ExternalOutput')

    tc_obj = tile.TileContext(nc, trace_sim=bool(int(os.environ.get('KB_SIMTRACE', '0'))))
    with tc_obj as tc:
        with (
            tc.tile_pool(name='consts', bufs=1) as consts,
            tc.tile_pool(name='wpool', bufs=1) as wpool,
            tc.tile_pool(name='acts', bufs=1) as acts,
            tc.tile_pool(name='uptmps', bufs=6) as uptmps,
            tc.tile_pool(name='psum', bufs=4, space='PSUM') as psum,
        ):
            xin = consts.tile([54, NB, T0], F32R)
            nc.gpsimd.dma_start(xin[:], x_d[:].rearrange('p (n t) -> p n t', t=T0))

            cur = [xin]           # list of act tiles
            cur_lay = layout(3, xin=True)
            cur_T = T0

            for b in range(nblocks):
                c = BLOCKS[b]
                if c['kind'] == 'up':
                    R = R3 if c['scale'] == 3 else R2
                    T_in, T_out = c['T_in'], c['T_out']
                    s_l, tiles_l = cur_lay
                    out_tiles = []
                    for i, (v0, nv, Pi) in enumerate(tiles_l):
                        ot = acts.tile([128, NB, T_out], F32R, tag=f'act{b % 2}_{i}', name=f'up{b}_{i}')
                        src = cur[i]
                        if src.dtype == F32R:
                            src = src[:, :, :].bitcast(F32)
                        if c['scale'] == 2:
                            # t'=0 and t'=11 plain copies (on ACT to offload DVE)
                            nc.scalar.copy(ot[0:Pi, :, 0], src[0:Pi, :, 0])
                            nc.scalar.copy(ot[0:Pi, :, T_out - 1], src[0:Pi, :, T_in - 1])
                            # odd outputs 1,3,..,9: 0.75*x[k] + 0.25*x[k+1], k=0..4
                            # even outputs 2,4,..,10: 0.25*x[k] + 0.75*x[k+1]
                            tmp = uptmps.tile([128, NB, T_in - 1], F32, tag='uptmp',
                                              name=f'uptmp{b}_{i}_a')
                            nc.vector.tensor_scalar_mul(
                                tmp[0:Pi], src[0:Pi, :, 1:T_in], 0.25)
                            nc.vector.scalar_tensor_tensor(
                                ot[0:Pi, :, 1:T_out - 1:2], src[0:Pi, :, 0:T_in - 1], 0.75,
                                tmp[0:Pi], mybir.AluOpType.mult, mybir.AluOpType.add)
                            tmp2 = uptmps.tile([128, NB, T_in - 1], F32, tag='uptmp2',
                                               name=f'uptmp{b}_{i}_b')
                            nc.vector.tensor_scalar_mul(
                                tmp2[0:Pi], src[0:Pi, :, 0:T_in - 1], 0.25)
                            nc.vector.scalar_tensor_tensor(
                                ot[0:Pi, :, 2:T_out:2], src[0:Pi, :, 1:T_in], 0.75,
                                tmp2[0:Pi], mybir.AluOpType.mult, mybir.AluOpType.add)
                        else:
                            # scale 3, T 2->6: t'0,1 <- x0; t'4,5 <- x1; t'2,3 blends
                            nc.scalar.copy(ot[0:Pi, :, 0], src[0:Pi, :, 0])
                            nc.scalar.copy(ot[0:Pi, :, 1], src[0:Pi, :, 0])
                            nc.scalar.copy(ot[0:Pi, :, 4], src[0:Pi, :, 1])
                            nc.scalar.copy(ot[0:Pi, :, 5], src[0:Pi, :, 1])
                            tmp = uptmps.tile([128, NB, 2], F32, tag='uptmp',
                                              name=f'uptmp{b}_{i}_a')
                            # columns 2,3: (2/3 x0 + 1/3 x1), (1/3 x0 + 2/3 x1)
                            w23 = float(1.0 / 3.0)
                            nc.vector.tensor_scalar_mul(tmp[0:Pi, :, 0], src[0:Pi, :, 1], w23)
                            nc.vector.tensor_scalar_mul(tmp[0:Pi, :, 1], src[0:Pi, :, 0], w23)
                            nc.vector.scalar_tensor_tensor(
                                ot[0:Pi, :, 2], src[0:Pi, :, 0], float(2.0 / 3.0),
                                tmp[0:Pi, :, 0], mybir.AluOpType.mult, mybir.AluOpType.add)
                            nc.vector.scalar_tensor_tensor(
                                ot[0:Pi, :, 3], src[0:Pi, :, 1], float(2.0 / 3.0),
                                tmp[0:Pi, :, 1], mybir.AluOpType.mult, mybir.AluOpType.add)
                        out_tiles.append(ot)
                    cur = out_tiles
                    cur_T = T_out
                    continue

                # conv block
                T_in, T_out, stride = c['T_in'], c['T_out'], c['stride']
                s_in, in_tiles = c['in_lay']
                s_out, out_tiles_l = c['out_lay']
                n_out = len(out_tiles_l)
                gdt = c['gcn_dt']
                rdt = c['res_dt']
                odt = c['act_out_dt']

                wbig_sh = dram_shapes[f'wbig{b}'][0]
                wbig = wpool.tile([128, max(wbig_sh[1], 1)], gdt, tag=f'wbig{b % 3}')
                if wbig_sh[1] > 0:
                    nc.gpsimd.dma_start(wbig[:, 0:wbig_sh[1]], w_d[f'wbig{b}'][:])
                wtcn = wpool.tile([128, 9 * 128], BF16, tag=f'wtcn{b % 3}')
                nc.gpsimd.dma_start(wtcn[:], w_d[f'wtcn{b}'][:])
                bias_sb = wpool.tile([128, n_out], F32, tag=f'bias{b % 3}')
                nc.gpsimd.dma_start(bias_sb[:], w_d[f'bias{b}'][:])
                beta_sb = wpool.tile([128, n_out], F32, tag=f'beta{b % 3}')
                nc.gpsimd.dma_start(beta_sb[:], w_d[f'beta{b}'][:])
                if c['res'] is not None:
                    wres_sh = dram_shapes[f'wres{b}'][0]
                    wres = wpool.tile([128, max(wres_sh[1], 1)], rdt, tag=f'wres{b % 3}')
                    nc.gpsimd.dma_start(wres[:, 0:wres_sh[1]], w_d[f'wres{b}'][:])

                nchunk = 32 if max(T_in, T_out) >= 12 else NB
                nck = NB // nchunk

                gact = [acts.tile([128, NB, T_in], BF16, tag=f'gact{b % 2}_{i}', name=f'gact{b}_{i}')
                        for i in range(n_out)]
                oact = [acts.tile([128, NB, T_out], odt, tag=f'act{b % 2}_{i}', name=f'oact{b}_{i}')
                        for i in range(n_out)]

                # group gcn calls by output tile
                by_j = {}
                for (i, j, ofs, ncols) in c['gcn_calls']:
                    by_j.setdefault(j, []).append((i, ofs))
                res_by_j = {}
                if c['res'] is not None:
                    for (i, j, ofs, ncols) in c['res_calls']:
                        res_by_j.setdefault(j, []).append((i, ofs))

                for ck in range(nck):
                    n0, n1 = ck * nchunk, (ck + 1) * nchunk
                    # --- gcn (+A, +bn1, +relu) ---
                    for j in range(n_out):
                        w0j, nwj, Pj = out_tiles_l[j]
                        ps = psum.tile([128, nchunk, T_in], F32, tag='ps', bufs=8, name=f'psg{b}_{ck}_{j}')
                        lst = by_j.get(j, [])
                        for q, (i, ofs) in enumerate(lst):
                            v0i, nvi, Pi = in_tiles[i]
                            nc.tensor.matmul(
                                ps[0:Pj, :, :],
                                wbig[0:Pi, ofs:ofs + Pj],
                                cur[i][0:Pi, n0:n1, :],
                                start=(q == 0), stop=(q == len(lst) - 1))
                        if j % 2 == 0:
                            nc.scalar.activation(
                                gact[j][0:Pj, n0:n1, :], ps[0:Pj, :, :],
                                mybir.ActivationFunctionType.Relu,
                                bias=bias_sb[0:Pj, j:j + 1])
                        else:
                            nc.vector.tensor_scalar(
                                gact[j][0:Pj, n0:n1, :], ps[0:Pj, :, :],
                                bias_sb[0:Pj, j:j + 1], 0.0,
                                mybir.AluOpType.add, mybir.AluOpType.max)
                # --- tcn (+res, +bn2, +beta, +out_act) ---
                for ck in range(nck):
                    n0, n1 = ck * nchunk, (ck + 1) * nchunk
                    for j in range(n_out):
                        w0j, nwj, Pj = out_tiles_l[j]
                        ps2 = psum.tile([128, nchunk, T_out], F32, tag='ps', bufs=8, name=f'pst{b}_{ck}_{j}')
                        dts = []
                        for dt in range(9):
                            t_lo = max(0, -(-(4 - dt) // stride))
                            # smallest t' with stride*t'+dt-4 >= 0
                            t_lo = max(0, (4 - dt + stride - 1) // stride)
                            # largest t' with stride*t'+dt-4 <= T_in-1
                            t_hi = min(T_out, (T_in - 1 - dt + 4) // stride + 1)
                            if t_hi > t_lo:
                                dts.append((dt, t_lo, t_hi))
                        # center tap first covers full range (start=True)
                        dts.sort(key=lambda z: -(z[2] - z[1]))
                        assert dts[0][1] == 0 and dts[0][2] == T_out, (b, dts)
                        n_res = len(res_by_j.get(j, [])) if c['res'] == 'conv' else 0
                        n_calls = len(dts) + n_res
                        q = 0
                        for (dt, t_lo, t_hi) in dts:
                            in_lo = stride * t_lo + dt - 4
                            nc.tensor.matmul(
                                ps2[0:Pj, :, t_lo:t_hi],
                                wtcn[0:Pj, dt * 128: dt * 128 + Pj],
                                gact[j][0:Pj, n0:n1, in_lo: in_lo + (t_hi - t_lo - 1) * stride + 1: stride]
                                if stride > 1 else
                                gact[j][0:Pj, n0:n1, in_lo: in_lo + (t_hi - t_lo)],
                                start=(q == 0), stop=(q == n_calls - 1))
                            q += 1
                        if c['res'] != 'id':
                            for (i, ofs) in res_by_j.get(j, []):
                                v0i, nvi, Pi = in_tiles[i]
                                rhs = cur[i][0:Pi, n0:n1, ::stride] if stride > 1 \
                                    else cur[i][0:Pi, n0:n1, :]
                                nc.tensor.matmul(
                                    ps2[0:Pj, :, :], wres[0:Pi, ofs:ofs + Pj], rhs,
                                    start=False, stop=(q == n_calls - 1))
                                q += 1
                        if c['res'] == 'id':
                            xres = cur[j][0:Pj, n0:n1, :]
                            if xres.dtype == F32R:
                                xres = xres.bitcast(F32)
                            nc.vector.scalar_tensor_tensor(
                                oact[j][0:Pj, n0:n1, :], ps2[0:Pj, :, :],
                                beta_sb[0:Pj, j:j + 1], xres,
                                mybir.AluOpType.add, mybir.AluOpType.add)
                            nc.scalar.activation(
                                oact[j][0:Pj, n0:n1, :],
                                oact[j][0:Pj, n0:n1, :] if oact[j].dtype != F32R
                                else oact[j][0:Pj, n0:n1, :].bitcast(F32),
                                mybir.ActivationFunctionType.Relu)
                        elif j % 2 == 0 or not c['out_act']:
                            func = (mybir.ActivationFunctionType.Relu if c['out_act']
                                    else mybir.ActivationFunctionType.Identity)
                            nc.scalar.activation(
                                oact[j][0:Pj, n0:n1, :], ps2[0:Pj, :, :], func,
                                bias=beta_sb[0:Pj, j:j + 1])
                        else:
                            nc.vector.tensor_scalar(
                                oact[j][0:Pj, n0:n1, :], ps2[0:Pj, :, :],
                                beta_sb[0:Pj, j:j + 1], 0.0,
                                mybir.AluOpType.add, mybir.AluOpType.max)

                cur = oact
                cur_lay = c['out_lay']
                cur_T = T_out

            # write output
            if nblocks == len(BLOCKS):
                s_o, tiles_o = cur_lay
                if len(tiles_o) == 1 and s_o == 3:
                    nc.gpsimd.dma_start(
                        y_d[:, :].rearrange('p (n t) -> p n t', t=T0),
                        cur[0][0:54, :, :])
                else:
                    for v in range(V):
                        ti = None
                        for i, (v0, nv, Pi) in enumerate(tiles_o):
                            if v0 <= v < v0 + nv:
                                ti = i
                                ro = (v - v0) * s_o
                        nc.gpsimd.dma_start(
                            y_d[v * 3:(v + 1) * 3, :].rearrange('p (n t) -> p n t', t=T0),
                            cur[ti][ro:ro + 3, :, :])
            else:
                s_o, tiles_o = cur_lay
                for i, (v0, nv, Pi) in enumerate(tiles_o):
                    src = cur[i][0:128, :, :]
                    if src.dtype == F32R:
                        src = src.bitcast(F32)
                    tmp = acts.tile([128, NB, cur_T], F32, tag=f'dbgout_{i}', name=f'dbg_{i}')
                    nc.vector.tensor_copy(tmp[:], src)
                    src = tmp[:]
                    nc.gpsimd.dma_start(
                        y_d[i * 128:(i + 1) * 128, :].rearrange(
                            'p (n t) -> p n t', t=cur_T), src)

    nc._kb_perfetto = getattr(tc_obj, '_perfetto_entries', None)
    nc.compile()
    return nc


def kernel(x, A, params):
    x = np.asarray(x, np.float32)
    dram = prep_weights(A, params)
    # cache key: the sparsity call pattern + shapes (the compiled program
    # depends on which Wbig blocks are nonzero, not on the weight values)
    key = (tuple(
        (b_i, tuple(c.get('gcn_calls', ())), tuple(c.get('res_calls', ())))
        for b_i, c in enumerate(BLOCKS) if c['kind'] == 'conv'),
        tuple(sorted((k, v.shape, str(v.dtype)) for k, v in dram.items())))
    if key not in _CACHE:
        _CACHE.clear()
        _CACHE[key] = build_bass(dram)
    nc = _CACHE[key]

    # x -> per-core [54, NB*T0] layout: row v*3+c, col n*T0+t
    in_maps = []
    for core in range(NCORES):
        xc = x[core * NB:(core + 1) * NB]            # (NB, 3, 12, 18)
        xl = np.ascontiguousarray(xc.transpose(3, 1, 0, 2)).reshape(54, NB * T0)
        m = {'xin': xl.astype(np.float32)}
        m.update(dram)
        in_maps.append(m)

    import time as _time
    trace = bool(int(os.environ.get('KB_TRACE', '0')))
    _t0 = _time.time()
    try:
        res = run_bass_kernel_spmd(nc, in_maps, core_ids=list(range(NCORES)), trace=trace)
    except Exception:
        # transient device wedge: reset cores and retry once
        os.environ['NEURON_RT_RESET_CORES'] = '1'
        _time.sleep(2.0)
        res = run_bass_kernel_spmd(nc, in_maps, core_ids=list(range(NCORES)), trace=trace)
    kernel.last_run_wall_ns = int((_time.time() - _t0) * 1e9)
    if trace and res.exec_time_ns is not None:
        kernel.last_exec_time_ns = res.exec_time_ns
        kernel.last_results = res

    nblocks = int(os.environ.get('KB_NUM_BLOCKS', len(BLOCKS)))
    outs = []
    for core in range(NCORES):
        yc = res.results[core]['y']
        if nblocks == len(BLOCKS):
            # [54, NB*T0] -> (NB, 3, 12, 18)
            out = yc.reshape(18, 3, NB, T0).transpose(2, 1, 3, 0)
            outs.append(out)
        else:
            outs.append(yc)
    if nblocks == len(BLOCKS):
        return np.ascontiguousarray(np.concatenate(outs, axis=0).astype(np.float32))
    return np.stack(outs)
